# revision 1
# baseline (speedup 1.0000x reference)
"""Trainium2 Bass kernel for nn_GaussianSplattingDecoder.

Splat 2048 gaussians onto a 200x200x16 voxel grid (V=640000), then a tiny
per-voxel MLP.  Exploits the radius-3 interaction mask: gaussian means are
~N(0,1) while the grid spans +-40 in x/y, so only ~3% of voxel tiles
interact with any gaussian at all.

Strategy (8 NeuronCores, SPMD — one program, per-core data):
  - Voxel tiles of TW=160 contiguous voxels.  Host finds, per tile, the
    candidate gaussians (dist(mean, tile bbox) < 3), packs them into blocks
    of 128 with tile-centered quadratic-form coefficients so both
      A = 0.5*mahalanobis - ln(opacity)   and   B = squared distance
    are K=8 matmuls (features [x'^2 y'^2 z'^2 x' y' z' 1 0]).
  - Device, per (tile, block) unit:  w = exp(-A) * (B < 9);  then
    psum2[18, TW] += semT.T @ w  (semantics cols 0..16, col 17 = 1 -> ws).
  - Per-tile epilogue: r = 1/max(ws, 1e-6), occ = psum2[:17]*r (PE
    broadcast of r), MLP (relu(W1@occ+b1), W2@h+b2), PE transpose, DMA out.
  - Inactive voxels: output is the constant c0 = W2@relu(b1)+b2; each core
    writes a c0-filled (V/8, 17) buffer; active tiles are computed into
    slot-indexed buffers and scattered over the fill on the host.
  - Active tiles are bucketed into block-count classes {1,2,4,8,16} and
    distributed round-robin so every core runs the identical static
    schedule (dummy all-zero slots pad the remainder; they are numerically
    inert and their outputs are ignored).
"""

import math
import numpy as np
from ml_dtypes import bfloat16

import concourse.bass as bass
import concourse.bacc as bacc
import concourse.mybir as mybir
from concourse import tile
from concourse.bass_utils import run_bass_kernel_spmd

AF = mybir.ActivationFunctionType
ALU = mybir.AluOpType
F32 = mybir.dt.float32

OCC = (200, 200, 16)
V = OCC[0] * OCC[1] * OCC[2]
C = 17
R2 = 9.0
TW = 160           # voxels per tile
BLK = 128          # gaussians per block
N_CORES = 8
CLASSES = (1, 2, 4, 8, 16)
VPC = V // N_CORES  # voxels per core (fill slab)


# ----------------------------------------------------------------- host math
def _softplus64(x):
    return np.logaddexp(0.0, x.astype(np.float64))


def _log_sigmoid64(x):
    x = x.astype(np.float64)
    return np.where(x >= 0, -np.log1p(np.exp(-np.abs(x))),
                    x - np.log1p(np.exp(-np.abs(x))))


def _plan_and_pack(gaussian_props, voxel_coords):
    """Compute the sparse schedule and per-core packed inputs."""
    gp = np.asarray(gaussian_props, np.float32)[0]          # (N, 28)
    vc = np.asarray(voxel_coords, np.float32)               # (V, 3)
    means = gp[:, :3]
    scales = _softplus64(gp[:, 3:6]).astype(np.float32)
    inv_s = (1.0 / np.clip(scales * scales, 1e-6, None)).astype(np.float32)
    logop = _log_sigmoid64(gp[:, 10]).astype(np.float32)
    sem = gp[:, 11:11 + C]

    nt = V // TW
    vt = vc.reshape(nt, TW, 3)
    lo, hi = vt.min(1), vt.max(1)

    # candidate gaussians per tile: dist(mean, bbox) < 3
    tiles = []  # (tile_id, idx array)
    for s in range(0, nt, 1024):
        e = min(s + 1024, nt)
        cl = np.clip(means[None, :, :], lo[s:e, None, :], hi[s:e, None, :])
        d2 = ((cl - means[None, :, :]) ** 2).sum(-1)
        for i in range(e - s):
            idx = np.nonzero(d2[i] < R2)[0]
            if len(idx):
                tiles.append((s + i, idx))

    # bucket into classes, round-robin across cores
    by_class = {J: [] for J in CLASSES}
    for tid, idx in tiles:
        nb = (len(idx) + BLK - 1) // BLK
        J = next(c for c in CLASSES if c >= nb)
        by_class[J].append((tid, idx))
    counts = {J: (len(by_class[J]) + N_CORES - 1) // N_CORES for J in CLASSES}
    schedule = [(J, counts[J]) for J in CLASSES if counts[J] > 0]
    S = sum(cnt for _, cnt in schedule)          # slots per core
    U = sum(J * cnt for J, cnt in schedule)      # units per core

    feats = np.zeros((N_CORES, S, 8, TW), np.float32)
    lhs = np.zeros((N_CORES, U, 2, 8, BLK), np.float32)
    semt = np.zeros((N_CORES, U, BLK, C + 1), bfloat16)
    # (core, slot) -> tile_id for output scatter; -1 = dummy
    slot_tile = np.full((N_CORES, S), -1, np.int64)

    for core in range(N_CORES):
        sid = 0
        uid = 0
        for J, cnt in schedule:
            mine = by_class[J][core::N_CORES]
            for s in range(cnt):
                if s < len(mine):
                    tid, idx = mine[s]
                    slot_tile[core, sid] = tid
                    ctr = 0.5 * (lo[tid] + hi[tid])
                    x = vt[tid] - ctr[None, :]
                    feats[core, sid, 0:3] = (x * x).T
                    feats[core, sid, 3:6] = x.T
                    feats[core, sid, 6] = 1.0
                    m = means[idx] - ctr[None, :]
                    iv = inv_s[idx]
                    n = len(idx)
                    cA = np.zeros((8, J * BLK), np.float32)
                    cS = np.zeros((8, J * BLK), np.float32)
                    cA[0:3, :n] = (0.5 * iv).T
                    cA[3:6, :n] = (-iv * m).T
                    cA[6, :n] = 0.5 * (iv * m * m).sum(1) - logop[idx]
                    cA[6, n:] = 1e4     # padding: w = exp(-1e4) = 0
                    cS[0:3, :n] = 1.0
                    cS[3:6, :n] = (-2.0 * m).T
                    cS[6, :n] = (m * m).sum(1)
                    cS[6, n:] = 1e9     # padding: mask = 0
                    # col 0 = 1 (-> ws at psum partition 0, engine reads
                    # must start at partition 0/32/64/96), cols 1.. = sem
                    sT = np.zeros((J * BLK, C + 1), np.float32)
                    sT[:n, 0] = 1.0
                    sT[:n, 1:] = sem[idx]
                    for j in range(J):
                        lhs[core, uid + j, 0] = cA[:, j*BLK:(j+1)*BLK]
                        lhs[core, uid + j, 1] = cS[:, j*BLK:(j+1)*BLK]
                        semt[core, uid + j] = sT[j*BLK:(j+1)*BLK].astype(bfloat16)
                # dummy slots stay all-zero (w=1 but sem=ws=0 -> out=c0)
                sid += 1
                uid += J
    return {
        "schedule": schedule, "S": S, "U": U, "slot_tile": slot_tile,
        "feats": feats, "lhs": lhs, "semt": semt,
    }


# ------------------------------------------------------------- bass program
def _build_program(schedule, S, U):
    nc = bacc.Bacc("TRN2", target_bir_lowering=False, debug=False,
                   num_devices=N_CORES)

    def din(name, shape, dt=F32):
        return nc.dram_tensor(name, list(shape), dt, kind="ExternalInput").ap()

    def dout(name, shape):
        return nc.dram_tensor(name, list(shape), F32, kind="ExternalOutput").ap()

    BF16 = mybir.dt.bfloat16
    feats_d = din("feats", (S, 8, TW))
    lhs_d = din("lhs", (U, 2, 8, BLK))
    semt_d = din("semt", (U, BLK, C + 1), BF16)
    w1t_d = din("w1t", (C + 1, 2 * C))  # row 0 zero (ignores ws row of occ)
    b1_d = din("b1", (2 * C, 1))
    w2t_d = din("w2t", (2 * C, C))
    b2_d = din("b2", (C, 1))
    b2row_d = din("b2row", (1, C))
    eye_d = din("eye", (C, C))
    fill_d = dout("fill", (VPC, C))
    slots_d = dout("slots", (S, TW, C))

    FILL_F = VPC * C // 128           # fill free-dim per partition (10625)
    FILL_CH = 5                       # fill DMA chunks
    assert FILL_F % (C * FILL_CH) == 0

    with tile.TileContext(nc) as tc:
        with (
            tc.tile_pool(name="const", bufs=1) as constp,
            tc.tile_pool(name="fillp", bufs=1) as fillp,
            tc.tile_pool(name="featp", bufs=2) as featp,
            tc.tile_pool(name="lhsp", bufs=2) as lhsp,
            tc.tile_pool(name="semp", bufs=2) as semp,
            tc.tile_pool(name="wp", bufs=4) as wp,
            tc.tile_pool(name="ep", bufs=3) as ep,
            tc.tile_pool(name="psab", bufs=4, space="PSUM") as psab,
            tc.tile_pool(name="ps2", bufs=2, space="PSUM") as ps2p,
            tc.tile_pool(name="pse", bufs=2, space="PSUM") as psep,
        ):
            # constants
            w1t_s = constp.tile([C + 1, 2 * C], F32, tag="w1t")
            nc.sync.dma_start(w1t_s[:], w1t_d[:])
            b1_s = constp.tile([2 * C, 1], F32, tag="b1")
            nc.sync.dma_start(b1_s[:], b1_d[:])
            w2t_s = constp.tile([2 * C, C], F32, tag="w2t")
            nc.sync.dma_start(w2t_s[:], w2t_d[:])
            b2_s = constp.tile([C, 1], F32, tag="b2")
            nc.sync.dma_start(b2_s[:], b2_d[:])
            b2row_s = constp.tile([1, C], F32, tag="b2row")
            nc.sync.dma_start(b2row_s[:], b2row_d[:])
            eye_s = constp.tile([C, C], F32, tag="eye")
            nc.sync.dma_start(eye_s[:], eye_d[:])
            ones_s = constp.tile([1, 128], F32, tag="ones")
            nc.vector.memset(ones_s[:], 1.0)

            # c0 = W2 @ relu(b1) + b2, as a row vector
            h0_s = constp.tile([2 * C, 1], F32, tag="h0")
            nc.scalar.activation(h0_s[:], b1_s[:], AF.Relu)
            pc0 = psep.tile([1, C], F32, tag="pse")
            nc.tensor.matmul(pc0[:], h0_s[:], w2t_s[:], start=True, stop=True)
            c0row_s = constp.tile([1, C], F32, tag="c0row")
            nc.vector.tensor_tensor(c0row_s[:], pc0[:], b2row_s[:], op=ALU.add)

            # c0 fill of the whole per-core slab: broadcast c0 to all 128
            # partitions via PE, then replicate along the free dim
            pfill = psep.tile([128, C], F32, tag="pse")
            nc.tensor.matmul(pfill[:], ones_s[:, 0:128], c0row_s[:],
                             start=True, stop=True)
            f17_s = constp.tile([128, C], F32, tag="f17")
            nc.scalar.activation(f17_s[:], pfill[:], AF.Copy)
            fill_s = fillp.tile([128, FILL_F], F32, tag="fill")
            fill_flat = fill_d.flatten().rearrange("(p f) -> p f", p=128)
            fchunk = FILL_F // FILL_CH
            for i in range(FILL_CH):
                sl = slice(i * fchunk, (i + 1) * fchunk)
                nc.gpsimd.tensor_copy(
                    fill_s[:, sl].rearrange("p (k c) -> p k c", c=C),
                    f17_s[:].unsqueeze(1).broadcast_to([128, fchunk // C, C]),
                )
                nc.sync.dma_start(fill_flat[:, sl], fill_s[:, sl])

            # main sparse loop
            sid = 0
            uid = 0
            for J, cnt in schedule:
                for _ in range(cnt):
                    # feats replicated at partitions 0-7 and 32-39 so the A
                    # and B matmuls run concurrently in two PE row strips
                    # NOTE: SBUF-side DMA APs need the partition dim
                    # outermost, so strips load as separate DMAs
                    feats_s = featp.tile([40, TW], F32, tag="feats")
                    nc.sync.dma_start(feats_s[0:8, :], feats_d[sid])
                    nc.sync.dma_start(feats_s[32:40, :], feats_d[sid])
                    # one DMA per strip for all J units' coefficients:
                    # A-coeffs at partitions 0-7, B-coeffs at 32-39, unit j
                    # in free columns j*128..
                    lhs_s = lhsp.tile([40, J * BLK], F32, tag=f"lhs{J}")
                    nc.sync.dma_start(
                        lhs_s[0:8, :].rearrange("p (j f) -> p j f", f=BLK),
                        lhs_d[uid:uid + J, 0].transpose([1, 0, 2]))
                    nc.sync.dma_start(
                        lhs_s[32:40, :].rearrange("p (j f) -> p j f", f=BLK),
                        lhs_d[uid:uid + J, 1].transpose([1, 0, 2]))
                    semt_s = semp.tile([BLK, J * (C + 1)], BF16, tag=f"sem{J}")
                    nc.sync.dma_start(
                        semt_s[:].rearrange("p (j f) -> p j f", f=C + 1),
                        semt_d[uid:uid + J].transpose([1, 0, 2]))
                    p2 = ps2p.tile([C + 1, TW], F32, tag="ps2")
                    for j in range(J):
                        pa = psab.tile([BLK, TW], F32, tag="psab")
                        pb = psab.tile([BLK, TW], F32, tag="psab")
                        nc.tensor.matmul(pa[:], lhs_s[0:8, bass.ts(j, BLK)],
                                         feats_s[0:8, :],
                                         start=True, stop=True,
                                         tile_position=(0, 0))
                        nc.tensor.matmul(pb[:], lhs_s[32:40, bass.ts(j, BLK)],
                                         feats_s[32:40, :],
                                         start=True, stop=True,
                                         tile_position=(32, 0))
                        we_s = wp.tile([BLK, TW], BF16, tag="we")
                        nc.scalar.activation(we_s[:], pa[:], AF.Exp, scale=-1.0)
                        w_s = wp.tile([BLK, TW], BF16, tag="w")
                        nc.vector.scalar_tensor_tensor(
                            w_s[:], pb[:], float(R2), we_s[:],
                            op0=ALU.is_lt, op1=ALU.mult)
                        nc.tensor.matmul(p2[:], semt_s[:, bass.ts(j, C + 1)],
                                         w_s[:],
                                         start=(j == 0), stop=(j == J - 1))
                    # epilogue: ws is p2 row 0; normalize all 18 rows (row 0
                    # becomes ~1, ignored via the zero first row of w1t)
                    r_s = ep.tile([1, TW], F32, tag="r")
                    nc.vector.tensor_scalar_max(r_s[:], p2[0:1, :], 1e-6)
                    nc.vector.reciprocal_approx_fast(r_s[:], r_s[:])
                    pr = psep.tile([C + 1, TW], F32, tag="pse")
                    nc.tensor.matmul(pr[:], ones_s[:, 0:C + 1], r_s[:],
                                     start=True, stop=True)
                    rb_s = ep.tile([C + 1, TW], F32, tag="rb")
                    nc.scalar.activation(rb_s[:], pr[:], AF.Copy)
                    occ_s = ep.tile([C + 1, TW], F32, tag="occ")
                    nc.vector.tensor_tensor(occ_s[:], p2[:], rb_s[:],
                                            op=ALU.mult)
                    ph = psep.tile([2 * C, TW], F32, tag="pse")
                    nc.tensor.matmul(ph[:], w1t_s[:], occ_s[:],
                                     start=True, stop=True)
                    h_s = ep.tile([2 * C, TW], F32, tag="h")
                    nc.scalar.activation(h_s[:], ph[:], AF.Relu, bias=b1_s[:])
                    po = psep.tile([C, TW], F32, tag="pse")
                    nc.tensor.matmul(po[:], w2t_s[:], h_s[:],
                                     start=True, stop=True)
                    o_s = ep.tile([C, TW], F32, tag="o")
                    nc.scalar.activation(o_s[:], po[:], AF.Identity,
                                         bias=b2_s[:])
                    for v0 in range(0, TW, 128):
                        vn = min(128, TW - v0)
                        pt = psep.tile([128, C], F32, tag="pse")
                        nc.tensor.transpose(pt[:vn, :], o_s[:, v0:v0 + vn],
                                            eye_s[:])
                        ot_s = ep.tile([128, C], F32, tag="ot")
                        nc.scalar.activation(ot_s[:vn, :], pt[:vn, :], AF.Copy)
                        nc.sync.dma_start(slots_d[sid, v0:v0 + vn, :],
                                          ot_s[:vn, :])
                    sid += 1
                    uid += J
    return nc


# ---------------------------------------------------------------- execution
def _execute(nc, plan, W1, b1, W2, b2, trace=False, **kw):
    w1t = np.zeros((C + 1, 2 * C), np.float32)
    w1t[1:] = W1.T
    consts = {
        "w1t": w1t,
        "b1": b1.reshape(2 * C, 1).astype(np.float32),
        "w2t": np.ascontiguousarray(W2.T).astype(np.float32),
        "b2": b2.reshape(C, 1).astype(np.float32),
        "b2row": b2.reshape(1, C).astype(np.float32),
        "eye": np.eye(C, dtype=np.float32),
    }
    in_maps = []
    for core in range(N_CORES):
        m = dict(consts)
        m["feats"] = plan["feats"][core]
        m["lhs"] = plan["lhs"][core]
        m["semt"] = plan["semt"][core]
        in_maps.append(m)
    if not nc.is_finalized():
        nc.finalize()
    return run_bass_kernel_spmd(nc, in_maps, list(range(N_CORES)),
                                trace=trace, **kw)


def _assemble(plan, results):
    out = np.empty((V, C), np.float32)
    for core in range(N_CORES):
        out[core * VPC:(core + 1) * VPC] = results[core]["fill"]
    slot_tile = plan["slot_tile"]
    for core in range(N_CORES):
        slots = results[core]["slots"]
        for sid in range(plan["S"]):
            tid = slot_tile[core, sid]
            if tid >= 0:
                out[tid * TW:(tid + 1) * TW] = slots[sid]
    return out.reshape(1, OCC[0], OCC[1], OCC[2], C)


def run(inputs, trace=False, **kw):
    """Full pipeline; returns (output, BassKernelResults)."""
    gp = np.asarray(inputs["gaussian_props"], np.float32)
    plan = _plan_and_pack(gp, inputs["voxel_coords"])
    nc = _build_program(plan["schedule"], plan["S"], plan["U"])
    res = _execute(nc, plan,
                   np.asarray(inputs["W1"], np.float32),
                   np.asarray(inputs["b1"], np.float32),
                   np.asarray(inputs["W2"], np.float32),
                   np.asarray(inputs["b2"], np.float32),
                   trace=trace, **kw)
    out = _assemble(plan, res.results)
    return out, res


def kernel(**inputs) -> np.ndarray:
    out, _ = run(inputs)
    return out



# revision 6
# speedup vs baseline: 1.3065x; 1.3065x over previous
"""Trainium2 Bass kernel for nn_GaussianSplattingDecoder.

Splat 2048 gaussians onto a 200x200x16 voxel grid (V=640000), then a tiny
per-voxel MLP.  Exploits the radius-3 interaction mask: gaussian means are
~N(0,1) while the grid spans +-40 in x/y, so only ~3% of voxel tiles
interact with any gaussian at all.

Strategy (8 NeuronCores, SPMD — one program, per-core data):
  - Voxel tiles of TW=160 contiguous voxels.  Host finds, per tile, the
    candidate gaussians (dist(mean, tile bbox) < 3), packs them into blocks
    of 128 with tile-centered quadratic-form coefficients so both
      A = 0.5*mahalanobis - ln(opacity)   and   B = squared distance
    are K=8 matmuls (features [x'^2 y'^2 z'^2 x' y' z' 1 0]).
  - The A matmul runs single-pass float32r (~11-bit mantissa, same speed
    as bf16): tile-centering keeps coefficients small enough that exp(-A)
    stays accurate.  The B matmul must stay fp32: the mask compare
    (B < 9) flips for borderline voxels otherwise, and flipped
    large-scale gaussians produce O(1) occupancy errors.
  - Device, per (tile, block) unit:  w = exp(-A) * (B < 9);  then
    psum2[18, TW] += semT.T @ w  (semantics cols 0..16, col 17 = 1 -> ws).
  - Per-tile epilogue, engineered off the Scalar engine (exp is Scalar-
    bound): DVE computes r = 1/max(ws,1e-6); PE broadcasts r; DVE
    normalizes; MLP layer 1 takes b1 via the ~1-valued ws row of occ
    (w1t row 0 = b1); DVE applies relu; layer 2 takes b2 via a ones row
    appended to h (w2t row 34 = b2).  Output stays [17, TW]; the host
    transposes.
  - All inputs load in 5 large up-front DMAs (the per-slot DMA descriptor
    overhead on the sync engine was ~0.7us each).
  - Inactive voxels get the constant c0 = W2@relu(b1)+b2, filled
    host-side; active tiles are computed into slot-indexed buffers and
    scattered over the fill on the host.
  - Active tiles are bucketed into block-count classes {1,2,4,8,16} and
    distributed round-robin so every core runs the identical static
    schedule (dummy all-zero slots pad the remainder; they are numerically
    inert and their outputs are ignored).
"""

import numpy as np
from ml_dtypes import bfloat16

import concourse.bass as bass
import concourse.bacc as bacc
import concourse.mybir as mybir
from concourse import tile
from concourse.bass_utils import run_bass_kernel_spmd

AF = mybir.ActivationFunctionType
ALU = mybir.AluOpType
F32 = mybir.dt.float32
F32R = mybir.dt.float32r
BF16 = mybir.dt.bfloat16

OCC = (200, 200, 16)
V = OCC[0] * OCC[1] * OCC[2]
C = 17
R2 = 9.0
TW = 160           # voxels per tile
BLK = 128          # gaussians per block
N_CORES = 8
CLASSES = (1, 2, 4, 8, 16)
VPC = V // N_CORES


# ----------------------------------------------------------------- host math
def _softplus64(x):
    return np.logaddexp(0.0, x.astype(np.float64))


def _log_sigmoid64(x):
    x = x.astype(np.float64)
    return np.where(x >= 0, -np.log1p(np.exp(-np.abs(x))),
                    x - np.log1p(np.exp(-np.abs(x))))


def _plan_and_pack(gaussian_props, voxel_coords):
    """Compute the sparse schedule and per-core packed inputs."""
    gp = np.asarray(gaussian_props, np.float32)[0]          # (N, 28)
    vc = np.asarray(voxel_coords, np.float32)               # (V, 3)
    means = gp[:, :3]
    scales = _softplus64(gp[:, 3:6]).astype(np.float32)
    inv_s = (1.0 / np.clip(scales * scales, 1e-6, None)).astype(np.float32)
    logop = _log_sigmoid64(gp[:, 10]).astype(np.float32)
    sem = gp[:, 11:11 + C]

    nt = V // TW
    vt = vc.reshape(nt, TW, 3)
    lo, hi = vt.min(1), vt.max(1)

    # candidate gaussians per tile: dist(mean, bbox) < 3
    tiles = []  # (tile_id, idx array)
    for s in range(0, nt, 1024):
        e = min(s + 1024, nt)
        cl = np.clip(means[None, :, :], lo[s:e, None, :], hi[s:e, None, :])
        d2 = ((cl - means[None, :, :]) ** 2).sum(-1)
        for i in range(e - s):
            idx = np.nonzero(d2[i] < R2)[0]
            if len(idx):
                tiles.append((s + i, idx))

    # bucket into classes, round-robin across cores
    by_class = {J: [] for J in CLASSES}
    for tid, idx in tiles:
        nb = (len(idx) + BLK - 1) // BLK
        J = next(c for c in CLASSES if c >= nb)
        by_class[J].append((tid, idx))
    counts = {J: (len(by_class[J]) + N_CORES - 1) // N_CORES for J in CLASSES}
    schedule = [(J, counts[J]) for J in CLASSES if counts[J] > 0]
    S = sum(cnt for _, cnt in schedule)          # slots per core
    U = sum(J * cnt for J, cnt in schedule)      # units per core

    feats = np.zeros((N_CORES, S, 8, TW), np.float32)
    lhsa = np.zeros((N_CORES, U, 8, BLK), np.float32)
    lhsb = np.zeros((N_CORES, U, 8, BLK), np.float32)
    semt = np.zeros((N_CORES, U, BLK, C + 1), bfloat16)
    # (core, slot) -> tile_id for output scatter; -1 = dummy
    slot_tile = np.full((N_CORES, S), -1, np.int64)

    for core in range(N_CORES):
        sid = 0
        uid = 0
        for J, cnt in schedule:
            mine = by_class[J][core::N_CORES]
            for s in range(cnt):
                if s < len(mine):
                    tid, idx = mine[s]
                    slot_tile[core, sid] = tid
                    ctr = 0.5 * (lo[tid] + hi[tid])
                    x = vt[tid] - ctr[None, :]
                    feats[core, sid, 0:3] = (x * x).T
                    feats[core, sid, 3:6] = x.T
                    feats[core, sid, 6] = 1.0
                    m = means[idx] - ctr[None, :]
                    iv = inv_s[idx]
                    n = len(idx)
                    cA = np.zeros((8, J * BLK), np.float32)
                    cS = np.zeros((8, J * BLK), np.float32)
                    cA[0:3, :n] = (0.5 * iv).T
                    cA[3:6, :n] = (-iv * m).T
                    cA[6, :n] = 0.5 * (iv * m * m).sum(1) - logop[idx]
                    cA[6, n:] = 1e4     # padding: w = exp(-1e4) = 0
                    cS[0:3, :n] = 1.0
                    cS[3:6, :n] = (-2.0 * m).T
                    cS[6, :n] = (m * m).sum(1)
                    cS[6, n:] = 1e9     # padding: mask = 0
                    # col 0 = 1 (-> ws at psum partition 0, engine reads
                    # must start at partition 0/32/64/96), cols 1.. = sem
                    sT = np.zeros((J * BLK, C + 1), np.float32)
                    sT[:n, 0] = 1.0
                    sT[:n, 1:] = sem[idx]
                    for j in range(J):
                        lhsa[core, uid + j] = cA[:, j*BLK:(j+1)*BLK]
                        lhsb[core, uid + j] = cS[:, j*BLK:(j+1)*BLK]
                        semt[core, uid + j] = sT[j*BLK:(j+1)*BLK].astype(bfloat16)
                # dummy slots stay all-zero (w=1 but sem=ws=0 -> out=c0)
                sid += 1
                uid += J
    return {
        "schedule": schedule, "S": S, "U": U, "slot_tile": slot_tile,
        "feats": feats, "lhsa": lhsa, "lhsb": lhsb, "semt": semt,
    }


# ------------------------------------------------------------- bass program
def _build_program(schedule, S, U):
    nc = bacc.Bacc("TRN2", target_bir_lowering=False, debug=False,
                   num_devices=N_CORES)

    def din(name, shape, dt=F32):
        return nc.dram_tensor(name, list(shape), dt, kind="ExternalInput").ap()

    def dout(name, shape):
        return nc.dram_tensor(name, list(shape), F32, kind="ExternalOutput").ap()

    featsr_d = din("featsr", (S, 8, TW), F32R)
    featsf_d = din("featsf", (S, 8, TW), F32)
    lhsa_d = din("lhsa", (U, 8, BLK), F32R)
    lhsb_d = din("lhsb", (U, 8, BLK), F32)
    semt_d = din("semt", (U, BLK, C + 1), BF16)
    w1te_d = din("w1te", (C + 1, 2 * C), BF16)   # row 0 = b1 (occ ws row ~ 1)
    w2e_d = din("w2e", (2 * C, C), BF16)
    b2_d = din("b2", (C, 1))
    slots_d = dout("slots", (S, C, TW))          # host transposes to (TW, C)

    with tile.TileContext(nc) as tc:
        with (
            tc.tile_pool(name="const", bufs=1) as constp,
            tc.tile_pool(name="wp", bufs=4) as wp,
            tc.tile_pool(name="ep", bufs=3) as ep,
            tc.tile_pool(name="psab", bufs=4, space="PSUM") as psab,
            tc.tile_pool(name="ps2", bufs=2, space="PSUM") as ps2p,
            tc.tile_pool(name="pse", bufs=2, space="PSUM") as psep,
        ):
            # constants + all inputs in a few large DMAs
            w1te_s = constp.tile([C + 1, 2 * C], BF16, tag="w1te")
            nc.sync.dma_start(w1te_s[:], w1te_d[:])
            w2e_s = constp.tile([2 * C, C], BF16, tag="w2e")
            nc.sync.dma_start(w2e_s[:], w2e_d[:])
            b2_s = constp.tile([C, 1], F32, tag="b2")
            nc.sync.dma_start(b2_s[:], b2_d[:])
            ones_s = constp.tile([1, C + 1], BF16, tag="ones")
            nc.vector.memset(ones_s[:], 1.0)

            featsr_s = constp.tile([8, S * TW], F32R, tag="featsr")
            nc.sync.dma_start(
                featsr_s[:].rearrange("p (s f) -> p s f", f=TW),
                featsr_d.transpose([1, 0, 2]))
            featsf_s = constp.tile([8, S * TW], F32, tag="featsf")
            nc.sync.dma_start(
                featsf_s[:].rearrange("p (s f) -> p s f", f=TW),
                featsf_d.transpose([1, 0, 2]))
            lhsa_s = constp.tile([8, U * BLK], F32R, tag="lhsa")
            nc.sync.dma_start(
                lhsa_s[:].rearrange("p (u f) -> p u f", f=BLK),
                lhsa_d.transpose([1, 0, 2]))
            lhsb_s = constp.tile([8, U * BLK], F32, tag="lhsb")
            nc.sync.dma_start(
                lhsb_s[:].rearrange("p (u f) -> p u f", f=BLK),
                lhsb_d.transpose([1, 0, 2]))
            semt_s = constp.tile([BLK, U * (C + 1)], BF16, tag="semt")
            nc.sync.dma_start(
                semt_s[:].rearrange("p (u f) -> p u f", f=C + 1),
                semt_d.transpose([1, 0, 2]))

            # main sparse loop
            sid = 0
            uid = 0
            for J, cnt in schedule:
                for _ in range(cnt):
                    p2 = ps2p.tile([C + 1, TW], F32, tag="ps2")
                    fsl = bass.ts(sid, TW)
                    for j in range(J):
                        usl = bass.ts(uid + j, BLK)
                        pa = psab.tile([BLK, TW], F32, tag="psab")
                        pb = psab.tile([BLK, TW], F32, tag="psab")
                        nc.tensor.matmul(pa[:], lhsa_s[:, usl],
                                         featsr_s[:, fsl],
                                         start=True, stop=True)
                        nc.tensor.matmul(pb[:], lhsb_s[:, usl],
                                         featsf_s[:, fsl],
                                         start=True, stop=True)
                        we_s = wp.tile([BLK, TW], BF16, tag="we")
                        nc.scalar.activation(we_s[:], pa[:], AF.Exp, scale=-1.0)
                        w_s = wp.tile([BLK, TW], BF16, tag="w")
                        nc.vector.scalar_tensor_tensor(
                            w_s[:], pb[:], float(R2), we_s[:],
                            op0=ALU.is_lt, op1=ALU.mult)
                        nc.tensor.matmul(p2[:],
                                         semt_s[:, bass.ts(uid + j, C + 1)],
                                         w_s[:],
                                         start=(j == 0), stop=(j == J - 1))
                    # epilogue: ws is p2 row 0; r broadcast via PE; the
                    # normalized ws row (~1) carries b1 through w1te row 0
                    r_s = ep.tile([1, TW], F32, tag="r")
                    nc.vector.tensor_scalar_max(r_s[:], p2[0:1, :], 1e-6)
                    nc.vector.reciprocal_approx_fast(r_s[:], r_s[:])
                    rc_s = ep.tile([1, TW], BF16, tag="rc")
                    nc.vector.tensor_scalar_mul(rc_s[:], r_s[:], 1.0)
                    pr = psep.tile([C + 1, TW], F32, tag="pse")
                    nc.tensor.matmul(pr[:], ones_s[:], rc_s[:],
                                     start=True, stop=True)
                    rb_s = ep.tile([C + 1, TW], F32, tag="rb")
                    nc.scalar.activation(rb_s[:], pr[:], AF.Copy)
                    occ_s = ep.tile([C + 1, TW], BF16, tag="occ")
                    nc.vector.tensor_tensor(occ_s[:], p2[:], rb_s[:],
                                            op=ALU.mult)
                    ph = psep.tile([2 * C, TW], F32, tag="pse")
                    nc.tensor.matmul(ph[:], w1te_s[:], occ_s[:],
                                     start=True, stop=True)
                    h_s = ep.tile([2 * C, TW], BF16, tag="h")
                    nc.vector.tensor_scalar_max(h_s[:], ph[:], 0.0)
                    po = psep.tile([C, TW], F32, tag="pse")
                    nc.tensor.matmul(po[:], w2e_s[:], h_s[:],
                                     start=True, stop=True)
                    o_s = ep.tile([C, TW], F32, tag="o")
                    nc.scalar.activation(o_s[:], po[:], AF.Identity,
                                         bias=b2_s[:])
                    nc.sync.dma_start(slots_d[sid], o_s[:])
                    sid += 1
                    uid += J
    return nc


# ---------------------------------------------------------------- execution
def _execute(nc, plan, W1, b1, W2, b2, trace=False, **kw):
    w1te = np.zeros((C + 1, 2 * C), np.float32)
    w1te[0] = b1
    w1te[1:] = W1.T
    consts = {
        "w1te": w1te.astype(bfloat16),
        "w2e": np.ascontiguousarray(W2.T).astype(bfloat16),
        "b2": b2.reshape(C, 1).astype(np.float32),
    }
    in_maps = []
    for core in range(N_CORES):
        m = dict(consts)
        m["featsr"] = plan["feats"][core]
        m["featsf"] = plan["feats"][core]
        m["lhsa"] = plan["lhsa"][core]
        m["lhsb"] = plan["lhsb"][core]
        m["semt"] = plan["semt"][core]
        in_maps.append(m)
    if not nc.is_finalized():
        nc.finalize()
    return run_bass_kernel_spmd(nc, in_maps, list(range(N_CORES)),
                                trace=trace, **kw)


def _assemble(plan, results, W1, b1, W2, b2):
    # inactive voxels: occ = 0 -> out = W2 @ relu(b1) + b2
    c0 = (W2 @ np.maximum(b1, 0.0) + b2).astype(np.float32)
    out = np.empty((V, C), np.float32)
    out[:] = c0
    slot_tile = plan["slot_tile"]
    for core in range(N_CORES):
        slots = results[core]["slots"]          # (S, C, TW)
        for sid in range(plan["S"]):
            tid = slot_tile[core, sid]
            if tid >= 0:
                out[tid * TW:(tid + 1) * TW] = slots[sid].T
    return out.reshape(1, OCC[0], OCC[1], OCC[2], C)


def run(inputs, trace=False, **kw):
    """Full pipeline; returns (output, BassKernelResults)."""
    gp = np.asarray(inputs["gaussian_props"], np.float32)
    plan = _plan_and_pack(gp, inputs["voxel_coords"])
    nc = _build_program(plan["schedule"], plan["S"], plan["U"])
    W1 = np.asarray(inputs["W1"], np.float32)
    b1 = np.asarray(inputs["b1"], np.float32)
    W2 = np.asarray(inputs["W2"], np.float32)
    b2 = np.asarray(inputs["b2"], np.float32)
    res = _execute(nc, plan, W1, b1, W2, b2, trace=trace, **kw)
    out = _assemble(plan, res.results, W1, b1, W2, b2)
    return out, res


def kernel(**inputs) -> np.ndarray:
    out, _ = run(inputs)
    return out


# revision 7
# speedup vs baseline: 1.5945x; 1.2204x over previous
"""Trainium2 Bass kernel for nn_GaussianSplattingDecoder.

Splat 2048 gaussians onto a 200x200x16 voxel grid (V=640000), then a tiny
per-voxel MLP.  Exploits the radius-3 interaction mask: gaussian means are
~N(0,1) while the grid spans +-40 in x/y, so only ~3% of voxel tiles
interact with any gaussian at all.

Strategy (8 NeuronCores, SPMD — one program, per-core data):
  - Voxel tiles of TW=160 contiguous voxels.  Host finds, per tile, the
    candidate gaussians (dist(mean, tile bbox) < 3), packs them into blocks
    of 128 with tile-centered quadratic-form coefficients so both
      A = 0.5*mahalanobis - ln(opacity)   and   B = squared distance
    are K=8 matmuls (features [x'^2 y'^2 z'^2 x' y' z' 1 0]).
  - Both matmuls run single-pass float32r (truncates operands to ~11
    mantissa bits, but at bf16 speed -- 4x faster than fp32's LOW_HIGH
    two-pass mode).  Because each tile sits at a single x value, only 5 of
    the 8 feature rows are needed (y'^2/z'^2 via exact small integer-grid
    features, y', z', 1); the 3 spare rows carry hi/lo-compensated
    coefficients: hi parts are pre-rounded to 9 mantissa bits (exact
    pass-through at the hardware's ~11-bit operand truncation, whatever
    its rounding mode), and the lo residuals ride the duplicated feature
    rows.  Net effective precision ~fp32; critical for the mask compare
    (B < 9), where ~1e-2 errors flip borderline voxels and large-scale
    gaussians then produce O(1) occupancy errors.
  - Device, per (tile, block) unit:  w = exp(-A) * (B < 9);  then
    psum2[18, TW] += semT.T @ w  (semantics cols 0..16, col 17 = 1 -> ws).
  - Per-tile epilogue, engineered off the Scalar engine (exp is Scalar-
    bound): DVE computes r = 1/max(ws,1e-6); PE broadcasts r; DVE
    normalizes; MLP layer 1 takes b1 via the ~1-valued ws row of occ
    (w1t row 0 = b1); DVE applies relu; layer 2 takes b2 via a ones row
    appended to h (w2t row 34 = b2).  Output stays [17, TW]; the host
    transposes.
  - All inputs load in 5 large up-front DMAs (the per-slot DMA descriptor
    overhead on the sync engine was ~0.7us each).
  - Inactive voxels get the constant c0 = W2@relu(b1)+b2, filled
    host-side; active tiles are computed into slot-indexed buffers and
    scattered over the fill on the host.
  - Active tiles are bucketed into block-count classes {1,2,4,8,16} and
    distributed round-robin so every core runs the identical static
    schedule (dummy all-zero slots pad the remainder; they are numerically
    inert and their outputs are ignored).
"""

import numpy as np
from ml_dtypes import bfloat16

import concourse.bass as bass
import concourse.bacc as bacc
import concourse.mybir as mybir
from concourse import tile
from concourse.bass_utils import run_bass_kernel_spmd

AF = mybir.ActivationFunctionType
ALU = mybir.AluOpType
F32 = mybir.dt.float32
F32R = mybir.dt.float32r
BF16 = mybir.dt.bfloat16

OCC = (200, 200, 16)
PCR_Y0, PCR_Y1 = -40.0, 40.0
PCR_Z0, PCR_Z1 = -1.0, 5.4
V = OCC[0] * OCC[1] * OCC[2]
C = 17
R2 = 9.0
TW = 160           # voxels per tile
BLK = 128          # gaussians per block
N_CORES = 8
CLASSES = (1, 2, 4, 8, 16)
VPC = V // N_CORES


# ----------------------------------------------------------------- host math
def _softplus64(x):
    return np.logaddexp(0.0, x.astype(np.float64))


def _log_sigmoid64(x):
    x = x.astype(np.float64)
    return np.where(x >= 0, -np.log1p(np.exp(-np.abs(x))),
                    x - np.log1p(np.exp(-np.abs(x))))


def _rne9(x):
    """Round to 9 explicit mantissa bits (exactly representable at the
    hardware's ~11-bit float32r operand truncation)."""
    x32 = np.asarray(x, np.float32)
    u = x32.view(np.uint32)
    out = ((u + np.uint32(1 << 13)) & np.uint32(0xFFFFC000)).view(np.float32)
    return np.where(np.isfinite(x32), out, x32).astype(np.float64)


def _plan_and_pack(gaussian_props, voxel_coords):
    """Compute the sparse schedule and per-core packed inputs."""
    gp = np.asarray(gaussian_props, np.float32)[0]          # (N, 28)
    vc = np.asarray(voxel_coords, np.float32)               # (V, 3)
    means = gp[:, :3].astype(np.float64)
    scales = _softplus64(gp[:, 3:6])
    inv_s = 1.0 / np.clip(scales * scales, 1e-6, None)
    logop = _log_sigmoid64(gp[:, 10])
    sem = gp[:, 11:11 + C]

    nt = V // TW
    vt = vc.reshape(nt, TW, 3)
    lo, hi = vt.min(1), vt.max(1)

    # candidate gaussians per tile: dist(mean, bbox) < 3
    m32 = gp[:, :3]
    tiles = []  # (tile_id, idx array)
    for s in range(0, nt, 1024):
        e = min(s + 1024, nt)
        cl = np.clip(m32[None, :, :], lo[s:e, None, :], hi[s:e, None, :])
        d2 = ((cl - m32[None, :, :]) ** 2).sum(-1)
        for i in range(e - s):
            idx = np.nonzero(d2[i] < R2)[0]
            if len(idx):
                tiles.append((s + i, idx))

    # bucket into classes, round-robin across cores
    by_class = {J: [] for J in CLASSES}
    for tid, idx in tiles:
        nb = (len(idx) + BLK - 1) // BLK
        J = next(c for c in CLASSES if c >= nb)
        by_class[J].append((tid, idx))
    counts = {J: (len(by_class[J]) + N_CORES - 1) // N_CORES for J in CLASSES}
    schedule = [(J, counts[J]) for J in CLASSES if counts[J] > 0]
    S = sum(cnt for _, cnt in schedule)          # slots per core
    U = sum(J * cnt for J, cnt in schedule)      # units per core

    featsa = np.zeros((N_CORES, S, 8, TW), np.float32)
    featsb = np.zeros((N_CORES, S, 8, TW), np.float32)
    lhsa = np.zeros((N_CORES, U, 8, BLK), np.float32)
    lhsb = np.zeros((N_CORES, U, 8, BLK), np.float32)
    semt = np.zeros((N_CORES, U, BLK, C + 1), bfloat16)
    # (core, slot) -> tile_id for output scatter; -1 = dummy
    slot_tile = np.full((N_CORES, S), -1, np.int64)

    # tile-local integer grid (tile = 10 y x 16 z at a single x)
    iy = np.arange(TW) // 16
    iz = np.arange(TW) % 16
    Fy = iy - 4.5                      # exact small values
    Fz = iz - 7.5
    dy = (PCR_Y1 - PCR_Y0) / (OCC[1] - 1)
    dz = (PCR_Z1 - PCR_Z0) / (OCC[2] - 1)

    for core in range(N_CORES):
        sid = 0
        uid = 0
        for J, cnt in schedule:
            mine = by_class[J][core::N_CORES]
            for s in range(cnt):
                if s < len(mine):
                    tid, idx = mine[s]
                    slot_tile[core, sid] = tid
                    ctr = (0.5 * (lo[tid].astype(np.float64)
                                  + hi[tid].astype(np.float64)))
                    yv = vt[tid][:, 1].astype(np.float64) - ctr[1]
                    zv = vt[tid][:, 2].astype(np.float64) - ctr[2]
                    y2z2 = yv * yv + zv * zv
                    fq_hi = _rne9(y2z2)
                    # features: A uses the exact integer-grid quadratics;
                    # B folds y'^2+z'^2 into a hi/lo pair (coeff 1)
                    featsa[core, sid] = np.stack([
                        Fy * Fy, Fz * Fz, Fy, Fz, np.ones(TW),
                        Fy, Fz, np.ones(TW)]).astype(np.float32)
                    featsb[core, sid] = np.stack([
                        fq_hi, y2z2 - fq_hi, Fy, Fz, np.ones(TW),
                        Fy, Fz, np.ones(TW)]).astype(np.float32)
                    m = means[idx] - ctr[None, :]
                    iv = inv_s[idx]
                    n = len(idx)
                    ay = 0.5 * iv[:, 1] * dy * dy
                    az = 0.5 * iv[:, 2] * dz * dz
                    by = -iv[:, 1] * m[:, 1] * dy
                    bz = -iv[:, 2] * m[:, 2] * dz
                    a0 = 0.5 * (iv[:, 1] * m[:, 1] ** 2
                                + iv[:, 2] * m[:, 2] ** 2
                                + iv[:, 0] * m[:, 0] ** 2) - logop[idx]
                    by_h = _rne9(by)
                    bz_h = _rne9(bz)
                    a0_h = _rne9(a0)
                    cA = np.zeros((8, J * BLK), np.float32)
                    cA[:, :n] = np.stack([ay, az, by_h, bz_h, a0_h,
                                          by - by_h, bz - bz_h, a0 - a0_h])
                    cA[4, n:] = 1e4     # padding: w = exp(-1e4) = 0
                    gy = -2.0 * m[:, 1] * dy
                    gz = -2.0 * m[:, 2] * dz
                    g0 = (m * m).sum(1)
                    gy_h = _rne9(gy)
                    gz_h = _rne9(gz)
                    g0_h = _rne9(g0)
                    cS = np.zeros((8, J * BLK), np.float32)
                    cS[:, :n] = np.stack([np.ones(n), np.ones(n), gy_h, gz_h,
                                          g0_h, gy - gy_h, gz - gz_h,
                                          g0 - g0_h])
                    cS[4, n:] = 1e9     # padding: mask = 0
                    # col 0 = 1 (-> ws at psum partition 0, engine reads
                    # must start at partition 0/32/64/96), cols 1.. = sem
                    sT = np.zeros((J * BLK, C + 1), np.float32)
                    sT[:n, 0] = 1.0
                    sT[:n, 1:] = sem[idx]
                    for j in range(J):
                        lhsa[core, uid + j] = cA[:, j*BLK:(j+1)*BLK]
                        lhsb[core, uid + j] = cS[:, j*BLK:(j+1)*BLK]
                        semt[core, uid + j] = sT[j*BLK:(j+1)*BLK].astype(bfloat16)
                # dummy slots stay all-zero (w=1 but sem=ws=0 -> out=c0)
                sid += 1
                uid += J
    return {
        "schedule": schedule, "S": S, "U": U, "slot_tile": slot_tile,
        "featsa": featsa, "featsb": featsb,
        "lhsa": lhsa, "lhsb": lhsb, "semt": semt,
    }


# ------------------------------------------------------------- bass program
def _build_program(schedule, S, U):
    nc = bacc.Bacc("TRN2", target_bir_lowering=False, debug=False,
                   num_devices=N_CORES)

    def din(name, shape, dt=F32):
        return nc.dram_tensor(name, list(shape), dt, kind="ExternalInput").ap()

    def dout(name, shape):
        return nc.dram_tensor(name, list(shape), F32, kind="ExternalOutput").ap()

    featsr_d = din("featsr", (S, 8, TW), F32R)
    featsf_d = din("featsf", (S, 8, TW), F32R)
    lhsa_d = din("lhsa", (U, 8, BLK), F32R)
    lhsb_d = din("lhsb", (U, 8, BLK), F32R)
    semt_d = din("semt", (U, BLK, C + 1), BF16)
    w1te_d = din("w1te", (C + 1, 2 * C), BF16)   # row 0 = b1 (occ ws row ~ 1)
    w2e_d = din("w2e", (2 * C, C), BF16)
    b2_d = din("b2", (C, 1))
    slots_d = dout("slots", (S, C, TW))          # host transposes to (TW, C)

    with tile.TileContext(nc) as tc:
        with (
            tc.tile_pool(name="const", bufs=1) as constp,
            tc.tile_pool(name="wp", bufs=4) as wp,
            tc.tile_pool(name="ep", bufs=3) as ep,
            tc.tile_pool(name="psab", bufs=4, space="PSUM") as psab,
            tc.tile_pool(name="ps2", bufs=2, space="PSUM") as ps2p,
            tc.tile_pool(name="pse", bufs=2, space="PSUM") as psep,
        ):
            # constants + all inputs in a few large DMAs
            w1te_s = constp.tile([C + 1, 2 * C], BF16, tag="w1te")
            nc.sync.dma_start(w1te_s[:], w1te_d[:])
            w2e_s = constp.tile([2 * C, C], BF16, tag="w2e")
            nc.sync.dma_start(w2e_s[:], w2e_d[:])
            b2_s = constp.tile([C, 1], F32, tag="b2")
            nc.sync.dma_start(b2_s[:], b2_d[:])
            ones_s = constp.tile([1, C + 1], BF16, tag="ones")
            nc.vector.memset(ones_s[:], 1.0)

            featsr_s = constp.tile([8, S * TW], F32R, tag="featsr")
            nc.sync.dma_start(
                featsr_s[:].rearrange("p (s f) -> p s f", f=TW),
                featsr_d.transpose([1, 0, 2]))
            featsf_s = constp.tile([8, S * TW], F32R, tag="featsf")
            nc.sync.dma_start(
                featsf_s[:].rearrange("p (s f) -> p s f", f=TW),
                featsf_d.transpose([1, 0, 2]))
            lhsa_s = constp.tile([8, U * BLK], F32R, tag="lhsa")
            nc.sync.dma_start(
                lhsa_s[:].rearrange("p (u f) -> p u f", f=BLK),
                lhsa_d.transpose([1, 0, 2]))
            lhsb_s = constp.tile([8, U * BLK], F32R, tag="lhsb")
            nc.sync.dma_start(
                lhsb_s[:].rearrange("p (u f) -> p u f", f=BLK),
                lhsb_d.transpose([1, 0, 2]))
            semt_s = constp.tile([BLK, U * (C + 1)], BF16, tag="semt")
            nc.sync.dma_start(
                semt_s[:].rearrange("p (u f) -> p u f", f=C + 1),
                semt_d.transpose([1, 0, 2]))

            # main sparse loop
            sid = 0
            uid = 0
            for J, cnt in schedule:
                for _ in range(cnt):
                    p2 = ps2p.tile([C + 1, TW], F32, tag="ps2")
                    fsl = bass.ts(sid, TW)
                    for j in range(J):
                        usl = bass.ts(uid + j, BLK)
                        pa = psab.tile([BLK, TW], F32, tag="psab")
                        pb = psab.tile([BLK, TW], F32, tag="psab")
                        nc.tensor.matmul(pa[:], lhsa_s[:, usl],
                                         featsr_s[:, fsl],
                                         start=True, stop=True)
                        nc.tensor.matmul(pb[:], lhsb_s[:, usl],
                                         featsf_s[:, fsl],
                                         start=True, stop=True)
                        we_s = wp.tile([BLK, TW], BF16, tag="we")
                        nc.scalar.activation(we_s[:], pa[:], AF.Exp, scale=-1.0)
                        w_s = wp.tile([BLK, TW], BF16, tag="w")
                        nc.vector.scalar_tensor_tensor(
                            w_s[:], pb[:], float(R2), we_s[:],
                            op0=ALU.is_lt, op1=ALU.mult)
                        nc.tensor.matmul(p2[:],
                                         semt_s[:, bass.ts(uid + j, C + 1)],
                                         w_s[:],
                                         start=(j == 0), stop=(j == J - 1))
                    # epilogue: ws is p2 row 0; r broadcast via PE; the
                    # normalized ws row (~1) carries b1 through w1te row 0
                    r_s = ep.tile([1, TW], F32, tag="r")
                    nc.vector.tensor_scalar_max(r_s[:], p2[0:1, :], 1e-6)
                    nc.vector.reciprocal_approx_fast(r_s[:], r_s[:])
                    rc_s = ep.tile([1, TW], BF16, tag="rc")
                    nc.vector.tensor_scalar_mul(rc_s[:], r_s[:], 1.0)
                    pr = psep.tile([C + 1, TW], F32, tag="pse")
                    nc.tensor.matmul(pr[:], ones_s[:], rc_s[:],
                                     start=True, stop=True)
                    rb_s = ep.tile([C + 1, TW], F32, tag="rb")
                    nc.scalar.activation(rb_s[:], pr[:], AF.Copy)
                    occ_s = ep.tile([C + 1, TW], BF16, tag="occ")
                    nc.vector.tensor_tensor(occ_s[:], p2[:], rb_s[:],
                                            op=ALU.mult)
                    ph = psep.tile([2 * C, TW], F32, tag="pse")
                    nc.tensor.matmul(ph[:], w1te_s[:], occ_s[:],
                                     start=True, stop=True)
                    h_s = ep.tile([2 * C, TW], BF16, tag="h")
                    nc.vector.tensor_scalar_max(h_s[:], ph[:], 0.0)
                    po = psep.tile([C, TW], F32, tag="pse")
                    nc.tensor.matmul(po[:], w2e_s[:], h_s[:],
                                     start=True, stop=True)
                    o_s = ep.tile([C, TW], F32, tag="o")
                    nc.scalar.activation(o_s[:], po[:], AF.Identity,
                                         bias=b2_s[:])
                    nc.sync.dma_start(slots_d[sid], o_s[:])
                    sid += 1
                    uid += J
    return nc


# ---------------------------------------------------------------- execution
def _execute(nc, plan, W1, b1, W2, b2, trace=False, **kw):
    w1te = np.zeros((C + 1, 2 * C), np.float32)
    w1te[0] = b1
    w1te[1:] = W1.T
    consts = {
        "w1te": w1te.astype(bfloat16),
        "w2e": np.ascontiguousarray(W2.T).astype(bfloat16),
        "b2": b2.reshape(C, 1).astype(np.float32),
    }
    in_maps = []
    for core in range(N_CORES):
        m = dict(consts)
        m["featsr"] = plan["featsa"][core]
        m["featsf"] = plan["featsb"][core]
        m["lhsa"] = plan["lhsa"][core]
        m["lhsb"] = plan["lhsb"][core]
        m["semt"] = plan["semt"][core]
        in_maps.append(m)
    if not nc.is_finalized():
        nc.finalize()
    return run_bass_kernel_spmd(nc, in_maps, list(range(N_CORES)),
                                trace=trace, **kw)


def _assemble(plan, results, W1, b1, W2, b2):
    # inactive voxels: occ = 0 -> out = W2 @ relu(b1) + b2
    c0 = (W2 @ np.maximum(b1, 0.0) + b2).astype(np.float32)
    out = np.empty((V, C), np.float32)
    out[:] = c0
    slot_tile = plan["slot_tile"]
    for core in range(N_CORES):
        slots = results[core]["slots"]          # (S, C, TW)
        for sid in range(plan["S"]):
            tid = slot_tile[core, sid]
            if tid >= 0:
                out[tid * TW:(tid + 1) * TW] = slots[sid].T
    return out.reshape(1, OCC[0], OCC[1], OCC[2], C)


def run(inputs, trace=False, **kw):
    """Full pipeline; returns (output, BassKernelResults)."""
    gp = np.asarray(inputs["gaussian_props"], np.float32)
    plan = _plan_and_pack(gp, inputs["voxel_coords"])
    nc = _build_program(plan["schedule"], plan["S"], plan["U"])
    W1 = np.asarray(inputs["W1"], np.float32)
    b1 = np.asarray(inputs["b1"], np.float32)
    W2 = np.asarray(inputs["W2"], np.float32)
    b2 = np.asarray(inputs["b2"], np.float32)
    res = _execute(nc, plan, W1, b1, W2, b2, trace=trace, **kw)
    out = _assemble(plan, res.results, W1, b1, W2, b2)
    return out, res


def kernel(**inputs) -> np.ndarray:
    out, _ = run(inputs)
    return out


# revision 10
# speedup vs baseline: 1.9904x; 1.2483x over previous
"""Trainium2 Bass kernel for nn_GaussianSplattingDecoder.

Splat 2048 gaussians onto a 200x200x16 voxel grid (V=640000), then a tiny
per-voxel MLP.  Exploits the radius-3 interaction mask: gaussian means are
~N(0,1) while the grid spans +-40 in x/y, so only ~3% of voxel tiles
interact with any gaussian at all.

Strategy (8 NeuronCores, SPMD — one program, per-core data):
  - Voxel tiles of TW=160 contiguous voxels.  Host finds, per tile, the
    candidate gaussians (dist(mean, tile bbox) < 3), packs them into blocks
    of 128 with tile-centered quadratic-form coefficients so both
      A = 0.5*mahalanobis - ln(opacity)   and   B = squared distance
    are K=8 matmuls (features [x'^2 y'^2 z'^2 x' y' z' 1 0]).
  - Both matmuls run single-pass float32r (truncates operands to ~11
    mantissa bits, but at bf16 speed -- 4x faster than fp32's LOW_HIGH
    two-pass mode).  Because each tile sits at a single x value, only 5 of
    the 8 feature rows are needed (y'^2/z'^2 via exact small integer-grid
    features, y', z', 1); the 3 spare rows carry hi/lo-compensated
    coefficients: hi parts are pre-rounded to 9 mantissa bits (exact
    pass-through at the hardware's ~11-bit operand truncation, whatever
    its rounding mode), and the lo residuals ride the duplicated feature
    rows.  Net effective precision ~fp32; critical for the mask compare
    (B < 9), where ~1e-2 errors flip borderline voxels and large-scale
    gaussians then produce O(1) occupancy errors.
  - Device, per (tile, block) unit:  w = exp(-A) * (B < 9);  then
    psum2[18, TW] += semT.T @ w  (semantics cols 0..16, col 17 = 1 -> ws).
  - Per-tile epilogue, engineered off the Scalar engine (exp is Scalar-
    bound): DVE computes r = 1/max(ws,1e-6); PE broadcasts r; DVE
    normalizes; MLP layer 1 takes b1 via the ~1-valued ws row of occ
    (w1t row 0 = b1); DVE applies relu; layer 2 takes b2 via a ones row
    appended to h (w2t row 34 = b2).  Output stays [17, TW]; the host
    transposes.
  - All inputs load in 5 large up-front DMAs (the per-slot DMA descriptor
    overhead on the sync engine was ~0.7us each).
  - Inactive voxels get the constant c0 = W2@relu(b1)+b2, filled
    host-side; active tiles are computed into slot-indexed buffers and
    scattered over the fill on the host.
  - Active tiles are bucketed into block-count classes {1,2,4,8,16} and
    distributed round-robin so every core runs the identical static
    schedule (dummy all-zero slots pad the remainder; they are numerically
    inert and their outputs are ignored).
"""

import numpy as np
from ml_dtypes import bfloat16

import concourse.bass as bass
import concourse.bacc as bacc
import concourse.mybir as mybir
from concourse import tile
from concourse.bass_utils import run_bass_kernel_spmd

AF = mybir.ActivationFunctionType
ALU = mybir.AluOpType
F32 = mybir.dt.float32
F32R = mybir.dt.float32r
BF16 = mybir.dt.bfloat16

OCC = (200, 200, 16)
PCR_Y0, PCR_Y1 = -40.0, 40.0
PCR_Z0, PCR_Z1 = -1.0, 5.4
V = OCC[0] * OCC[1] * OCC[2]
C = 17
R2 = 9.0
TW = 160           # voxels per tile
BLK = 128          # gaussians per block
N_CORES = 8
CLASSES = (1, 2, 4, 8, 16)
VPC = V // N_CORES


# ----------------------------------------------------------------- host math
def _softplus64(x):
    return np.logaddexp(0.0, x.astype(np.float64))


def _log_sigmoid64(x):
    x = x.astype(np.float64)
    return np.where(x >= 0, -np.log1p(np.exp(-np.abs(x))),
                    x - np.log1p(np.exp(-np.abs(x))))


def _rne9(x):
    """Round to 9 explicit mantissa bits (exactly representable at the
    hardware's ~11-bit float32r operand truncation)."""
    x32 = np.asarray(x, np.float32)
    u = x32.view(np.uint32)
    out = ((u + np.uint32(1 << 13)) & np.uint32(0xFFFFC000)).view(np.float32)
    return np.where(np.isfinite(x32), out, x32).astype(np.float64)


def _plan_and_pack(gaussian_props, voxel_coords):
    """Compute the sparse schedule and per-core packed inputs."""
    gp = np.asarray(gaussian_props, np.float32)[0]          # (N, 28)
    vc = np.asarray(voxel_coords, np.float32)               # (V, 3)
    means = gp[:, :3].astype(np.float64)
    scales = _softplus64(gp[:, 3:6])
    inv_s = 1.0 / np.clip(scales * scales, 1e-6, None)
    logop = _log_sigmoid64(gp[:, 10])
    sem = gp[:, 11:11 + C]

    nt = V // TW
    vt = vc.reshape(nt, TW, 3)
    lo, hi = vt.min(1), vt.max(1)

    # candidate gaussians per tile: dist(mean, bbox) < 3
    m32 = gp[:, :3]
    tiles = []  # (tile_id, idx array)
    for s in range(0, nt, 1024):
        e = min(s + 1024, nt)
        cl = np.clip(m32[None, :, :], lo[s:e, None, :], hi[s:e, None, :])
        d2 = ((cl - m32[None, :, :]) ** 2).sum(-1)
        for i in range(e - s):
            idx = np.nonzero(d2[i] < R2)[0]
            if len(idx):
                tiles.append((s + i, idx))

    # sort tiles by block count desc, group 8 at a time (one per core);
    # each group becomes one slot whose width J is the group's max nb
    tiles.sort(key=lambda t: -len(t[1]))
    groups = [tiles[g:g + N_CORES] for g in range(0, len(tiles), N_CORES)]
    schedule = [(len(grp[0][1]) + BLK - 1) // BLK for grp in groups]
    S = len(schedule)                            # slots per core
    U = sum(schedule)                            # units per core

    featsa = np.zeros((N_CORES, S, 8, TW), np.float32)
    featsb = np.zeros((N_CORES, S, 8, TW), np.float32)
    lhsa = np.zeros((N_CORES, U, 8, BLK), np.float32)
    lhsb = np.zeros((N_CORES, U, 8, BLK), np.float32)
    semt = np.zeros((N_CORES, U, BLK, C + 1), bfloat16)
    # (core, slot) -> tile_id for output scatter; -1 = dummy
    slot_tile = np.full((N_CORES, S), -1, np.int64)

    # tile-local integer grid (tile = 10 y x 16 z at a single x)
    iy = np.arange(TW) // 16
    iz = np.arange(TW) % 16
    Fy = iy - 4.5                      # exact small values
    Fz = iz - 7.5
    dy = (PCR_Y1 - PCR_Y0) / (OCC[1] - 1)
    dz = (PCR_Z1 - PCR_Z0) / (OCC[2] - 1)

    for core in range(N_CORES):
        uid = 0
        for sid, J in enumerate(schedule):
            grp = groups[sid]
            if True:
                if core < len(grp):
                    tid, idx = grp[core]
                    slot_tile[core, sid] = tid
                    ctr = (0.5 * (lo[tid].astype(np.float64)
                                  + hi[tid].astype(np.float64)))
                    yv = vt[tid][:, 1].astype(np.float64) - ctr[1]
                    zv = vt[tid][:, 2].astype(np.float64) - ctr[2]
                    y2z2 = yv * yv + zv * zv
                    fq_hi = _rne9(y2z2)
                    # features: A uses the exact integer-grid quadratics;
                    # B folds y'^2+z'^2 into a hi/lo pair (coeff 1)
                    featsa[core, sid] = np.stack([
                        Fy * Fy, Fz * Fz, Fy, Fz, np.ones(TW),
                        Fy, Fz, np.ones(TW)]).astype(np.float32)
                    featsb[core, sid] = np.stack([
                        fq_hi, y2z2 - fq_hi, Fy, Fz, np.ones(TW),
                        Fy, Fz, np.ones(TW)]).astype(np.float32)
                    m = means[idx] - ctr[None, :]
                    iv = inv_s[idx]
                    n = len(idx)
                    ay = 0.5 * iv[:, 1] * dy * dy
                    az = 0.5 * iv[:, 2] * dz * dz
                    by = -iv[:, 1] * m[:, 1] * dy
                    bz = -iv[:, 2] * m[:, 2] * dz
                    a0 = 0.5 * (iv[:, 1] * m[:, 1] ** 2
                                + iv[:, 2] * m[:, 2] ** 2
                                + iv[:, 0] * m[:, 0] ** 2) - logop[idx]
                    by_h = _rne9(by)
                    bz_h = _rne9(bz)
                    a0_h = _rne9(a0)
                    cA = np.zeros((8, J * BLK), np.float32)
                    cA[:, :n] = np.stack([ay, az, by_h, bz_h, a0_h,
                                          by - by_h, bz - bz_h, a0 - a0_h])
                    cA[4, n:] = 1e4     # padding: w = exp(-1e4) = 0
                    gy = -2.0 * m[:, 1] * dy
                    gz = -2.0 * m[:, 2] * dz
                    g0 = (m * m).sum(1)
                    gy_h = _rne9(gy)
                    gz_h = _rne9(gz)
                    g0_h = _rne9(g0)
                    cS = np.zeros((8, J * BLK), np.float32)
                    cS[:, :n] = np.stack([np.ones(n), np.ones(n), gy_h, gz_h,
                                          g0_h, gy - gy_h, gz - gz_h,
                                          g0 - g0_h])
                    cS[4, n:] = 1e9     # padding: mask = 0
                    # col 0 = 1 (-> ws at psum partition 0, engine reads
                    # must start at partition 0/32/64/96), cols 1.. = sem
                    sT = np.zeros((J * BLK, C + 1), np.float32)
                    sT[:n, 0] = 1.0
                    sT[:n, 1:] = sem[idx]
                    for j in range(J):
                        lhsa[core, uid + j] = cA[:, j*BLK:(j+1)*BLK]
                        lhsb[core, uid + j] = cS[:, j*BLK:(j+1)*BLK]
                        semt[core, uid + j] = sT[j*BLK:(j+1)*BLK].astype(bfloat16)
                # dummy slots stay all-zero (w=1 but sem=ws=0 -> out=c0)
                sid += 1
                uid += J
    return {
        "schedule": schedule, "S": S, "U": U, "slot_tile": slot_tile,
        "featsa": featsa, "featsb": featsb,
        "lhsa": lhsa, "lhsb": lhsb, "semt": semt,
    }


# ------------------------------------------------------------- bass program
def _build_program(schedule, S, U):
    nc = bacc.Bacc("TRN2", target_bir_lowering=False, debug=False,
                   num_devices=N_CORES)

    def din(name, shape, dt=F32):
        return nc.dram_tensor(name, list(shape), dt, kind="ExternalInput").ap()

    def dout(name, shape):
        return nc.dram_tensor(name, list(shape), F32, kind="ExternalOutput").ap()

    featsr_d = din("featsr", (S, 8, TW), F32R)
    featsf_d = din("featsf", (S, 8, TW), F32R)
    lhsa_d = din("lhsa", (U, 8, BLK), F32R)
    lhsb_d = din("lhsb", (U, 8, BLK), F32R)
    semt_d = din("semt", (U, BLK, C + 1), BF16)
    w1te_d = din("w1te", (C + 1, 2 * C), BF16)   # row 0 = b1 (occ ws row ~ 1)
    w2e_d = din("w2e", (2 * C, C), BF16)
    b2_d = din("b2", (C, 1))
    slots_d = dout("slots", (S, C, TW))          # host transposes to (TW, C)

    with tile.TileContext(nc) as tc:
        with (
            tc.tile_pool(name="const", bufs=1) as constp,
            tc.tile_pool(name="wp", bufs=4) as wp,
            tc.tile_pool(name="ep", bufs=3) as ep,
            tc.tile_pool(name="psa", bufs=2, space="PSUM") as psa,
            tc.tile_pool(name="psb", bufs=2, space="PSUM") as psb,
            tc.tile_pool(name="ps2", bufs=2, space="PSUM") as ps2p,
            tc.tile_pool(name="pse", bufs=2, space="PSUM") as psep,
        ):
            # constants + all inputs in a few large DMAs
            w1te_s = constp.tile([C + 1, 2 * C], BF16, tag="w1te")
            nc.sync.dma_start(w1te_s[:], w1te_d[:])
            w2e_s = constp.tile([2 * C, C], BF16, tag="w2e")
            nc.sync.dma_start(w2e_s[:], w2e_d[:])
            b2_s = constp.tile([C, 1], F32, tag="b2")
            nc.sync.dma_start(b2_s[:], b2_d[:])
            ones_s = constp.tile([1, C + 1], BF16, tag="ones")
            nc.vector.memset(ones_s[:], 1.0)

            featsr_s = constp.tile([8, S * TW], F32R, tag="featsr")
            nc.sync.dma_start(
                featsr_s[:].rearrange("p (s f) -> p s f", f=TW),
                featsr_d.transpose([1, 0, 2]))
            featsf_s = constp.tile([8, S * TW], F32R, tag="featsf")
            nc.sync.dma_start(
                featsf_s[:].rearrange("p (s f) -> p s f", f=TW),
                featsf_d.transpose([1, 0, 2]))
            lhsa_s = constp.tile([8, U * BLK], F32R, tag="lhsa")
            nc.sync.dma_start(
                lhsa_s[:].rearrange("p (u f) -> p u f", f=BLK),
                lhsa_d.transpose([1, 0, 2]))
            lhsb_s = constp.tile([8, U * BLK], F32R, tag="lhsb")
            nc.sync.dma_start(
                lhsb_s[:].rearrange("p (u f) -> p u f", f=BLK),
                lhsb_d.transpose([1, 0, 2]))
            semt_s = constp.tile([BLK, U * (C + 1)], BF16, tag="semt")
            nc.sync.dma_start(
                semt_s[:].rearrange("p (u f) -> p u f", f=C + 1),
                semt_d.transpose([1, 0, 2]))

            # main sparse loop: units processed in pairs sharing PSUM
            # banks, halving exp/STT instruction count
            units = []           # (sid, uid, first, last)
            uid = 0
            for sid, J in enumerate(schedule):
                for j in range(J):
                    units.append((sid, uid + j, j == 0, j == J - 1))
                uid += J

            def epilogue(sid, p2):
                # ws is p2 row 0; r broadcast via PE; the normalized ws row
                # (~1) carries b1 through w1te row 0
                r_s = ep.tile([1, TW], F32, tag="r")
                nc.vector.tensor_scalar_max(r_s[:], p2[0:1, :], 1e-6)
                nc.vector.reciprocal_approx_fast(r_s[:], r_s[:])
                rc_s = ep.tile([1, TW], BF16, tag="rc")
                nc.vector.tensor_scalar_mul(rc_s[:], r_s[:], 1.0)
                pr = psep.tile([C + 1, TW], F32, tag="pse")
                nc.tensor.matmul(pr[:], ones_s[:], rc_s[:],
                                 start=True, stop=True)
                rb_s = ep.tile([C + 1, TW], F32, tag="rb")
                nc.scalar.activation(rb_s[:], pr[:], AF.Copy)
                occ_s = ep.tile([C + 1, TW], BF16, tag="occ")
                nc.vector.tensor_tensor(occ_s[:], p2[:], rb_s[:],
                                        op=ALU.mult)
                ph = psep.tile([2 * C, TW], F32, tag="pse")
                nc.tensor.matmul(ph[:], w1te_s[:], occ_s[:],
                                 start=True, stop=True)
                h_s = ep.tile([2 * C, TW], BF16, tag="h")
                nc.vector.tensor_scalar_max(h_s[:], ph[:], 0.0)
                po = psep.tile([C, TW], F32, tag="pse")
                nc.tensor.matmul(po[:], w2e_s[:], h_s[:],
                                 start=True, stop=True)
                o_s = ep.tile([C, TW], F32, tag="o")
                nc.scalar.activation(o_s[:], po[:], AF.Identity,
                                     bias=b2_s[:])
                nc.sync.dma_start(slots_d[sid], o_s[:])

            p2_tiles = {}
            for p in range(0, len(units), 2):
                grp = units[p:p + 2]
                gw = len(grp) * TW
                pa2 = psa.tile([BLK, 2 * TW], F32, tag="pa")
                pb2 = psb.tile([BLK, 2 * TW], F32, tag="pb")
                for k, (sid, u, first, last) in enumerate(grp):
                    nc.tensor.matmul(pa2[:, bass.ts(k, TW)],
                                     lhsa_s[:, bass.ts(u, BLK)],
                                     featsr_s[:, bass.ts(sid, TW)],
                                     start=True, stop=True)
                for k, (sid, u, first, last) in enumerate(grp):
                    nc.tensor.matmul(pb2[:, bass.ts(k, TW)],
                                     lhsb_s[:, bass.ts(u, BLK)],
                                     featsf_s[:, bass.ts(sid, TW)],
                                     start=True, stop=True)
                we_s = wp.tile([BLK, 2 * TW], BF16, tag="we")
                nc.scalar.activation(we_s[:, 0:gw], pa2[:, 0:gw],
                                     AF.Exp, scale=-1.0)
                w_s = wp.tile([BLK, 2 * TW], BF16, tag="w")
                nc.vector.scalar_tensor_tensor(
                    w_s[:, 0:gw], pb2[:, 0:gw], float(R2), we_s[:, 0:gw],
                    op0=ALU.is_lt, op1=ALU.mult)
                for k, (sid, u, first, last) in enumerate(grp):
                    if first:
                        p2t = ps2p.tile([C + 1, TW], F32, tag="ps2")
                        p2_tiles[sid] = p2t
                    nc.tensor.matmul(p2_tiles[sid][:],
                                     semt_s[:, bass.ts(u, C + 1)],
                                     w_s[:, bass.ts(k, TW)],
                                     start=first, stop=last)
                    if last:
                        epilogue(sid, p2_tiles.pop(sid))
    return nc


# ---------------------------------------------------------------- execution
def _execute(nc, plan, W1, b1, W2, b2, trace=False, **kw):
    w1te = np.zeros((C + 1, 2 * C), np.float32)
    w1te[0] = b1
    w1te[1:] = W1.T
    consts = {
        "w1te": w1te.astype(bfloat16),
        "w2e": np.ascontiguousarray(W2.T).astype(bfloat16),
        "b2": b2.reshape(C, 1).astype(np.float32),
    }
    in_maps = []
    for core in range(N_CORES):
        m = dict(consts)
        m["featsr"] = plan["featsa"][core]
        m["featsf"] = plan["featsb"][core]
        m["lhsa"] = plan["lhsa"][core]
        m["lhsb"] = plan["lhsb"][core]
        m["semt"] = plan["semt"][core]
        in_maps.append(m)
    if not nc.is_finalized():
        nc.finalize()
    return run_bass_kernel_spmd(nc, in_maps, list(range(N_CORES)),
                                trace=trace, **kw)


def _assemble(plan, results, W1, b1, W2, b2):
    # inactive voxels: occ = 0 -> out = W2 @ relu(b1) + b2
    c0 = (W2 @ np.maximum(b1, 0.0) + b2).astype(np.float32)
    out = np.empty((V, C), np.float32)
    out[:] = c0
    slot_tile = plan["slot_tile"]
    for core in range(N_CORES):
        slots = results[core]["slots"]          # (S, C, TW)
        for sid in range(plan["S"]):
            tid = slot_tile[core, sid]
            if tid >= 0:
                out[tid * TW:(tid + 1) * TW] = slots[sid].T
    return out.reshape(1, OCC[0], OCC[1], OCC[2], C)


def run(inputs, trace=False, **kw):
    """Full pipeline; returns (output, BassKernelResults)."""
    gp = np.asarray(inputs["gaussian_props"], np.float32)
    plan = _plan_and_pack(gp, inputs["voxel_coords"])
    nc = _build_program(plan["schedule"], plan["S"], plan["U"])
    W1 = np.asarray(inputs["W1"], np.float32)
    b1 = np.asarray(inputs["b1"], np.float32)
    W2 = np.asarray(inputs["W2"], np.float32)
    b2 = np.asarray(inputs["b2"], np.float32)
    res = _execute(nc, plan, W1, b1, W2, b2, trace=trace, **kw)
    out = _assemble(plan, res.results, W1, b1, W2, b2)
    return out, res


def kernel(**inputs) -> np.ndarray:
    out, _ = run(inputs)
    return out


# revision 12
# speedup vs baseline: 2.4147x; 1.2132x over previous
"""Trainium2 Bass kernel for nn_GaussianSplattingDecoder.

Splat 2048 gaussians onto a 200x200x16 voxel grid (V=640000), then a tiny
per-voxel MLP.  Exploits the radius-3 interaction mask: gaussian means are
~N(0,1) while the grid spans +-40 in x/y, so only ~3% of voxel tiles
interact with any gaussian at all.

Strategy (8 NeuronCores, SPMD — one program, per-core data):
  - Voxel tiles of TW=160 contiguous voxels (10 y x 16 z at a single x).
    Host finds, per tile, the candidate gaussians (dist(mean, bbox) < 3),
    packs them into blocks of 128 with tile-centered quadratic-form
    coefficients so both
      A = 0.5*mahalanobis - ln(opacity)   and   B = squared distance
    are K=8 matmuls against shared per-voxel feature rows.
  - Both matmuls run single-pass float32r (the PE truncates operands to
    ~11 mantissa bits but runs at bf16 speed, 4x faster than fp32's
    LOW_HIGH two-pass mode).  Because x is constant per tile, only 5
    feature rows are needed (y'^2/z'^2 via exact small integer-grid
    features, y', z', 1); the 3 spare rows carry hi/lo-compensated
    coefficients: hi parts are pre-rounded to 9 mantissa bits (exact
    pass-through at the hardware's operand truncation, whatever its
    rounding mode) and the lo residuals ride duplicated feature rows.
    Net effective precision ~fp32 — critical for the mask compare
    (B < 9), where ~1e-2 errors flip borderline voxels and flipped
    large-scale gaussians produce O(1) occupancy errors.
  - Device, per (tile, block) unit:  w = exp(-A) * (B < 9);  then
    psum2[18, TW] += semT.T @ w  (semantics cols 0..16, col 17 = 1 -> ws).
    Units are processed in triples sharing one PSUM bank ([128, 480]) so
    exp/STT run as 3x-wide instructions (fixed per-op overhead dominates).
  - Per-tile epilogue, batched over pairs of slots ([18, 320] psum
    accumulators) and engineered off the Scalar engine: DVE computes
    r = 1/max(ws,1e-6); PE broadcasts r; DVE normalizes; MLP layer 1
    takes b1 via the ~1-valued ws row of occ (w1t row 0 = b1); DVE
    applies relu; b2 rides the scalar psum->sbuf copy.  Output stays
    [17, TW]; the host transposes.
  - All inputs load in a few large up-front DMAs (per-descriptor overhead
    on the sync queue is ~0.7us), chunked so the first units' data lands
    first.
  - Inactive voxels get the constant c0 = W2@relu(b1)+b2, filled
    host-side; active tiles are computed into slot-indexed buffers and
    scattered over the fill on the host.
  - Active tiles are sorted by block count and grouped 8 at a time (one
    per core, slot width = group max) so every core runs the identical
    static schedule; missing tiles become dummy all-zero slots which are
    numerically inert and ignored.
"""

import numpy as np
from ml_dtypes import bfloat16

import concourse.bass as bass
import concourse.bacc as bacc
import concourse.mybir as mybir
from concourse import tile
from concourse.bass_utils import run_bass_kernel_spmd

AF = mybir.ActivationFunctionType
ALU = mybir.AluOpType
F32 = mybir.dt.float32
F32R = mybir.dt.float32r
BF16 = mybir.dt.bfloat16

OCC = (200, 200, 16)
PCR_Y0, PCR_Y1 = -40.0, 40.0
PCR_Z0, PCR_Z1 = -1.0, 5.4
V = OCC[0] * OCC[1] * OCC[2]
C = 17
R2 = 9.0
TW = 160           # voxels per tile
BLK = 128          # gaussians per block
N_CORES = 8
GN = 3             # units per exp/STT group (PSUM bank holds 480 f32)
EB = 2             # slots per batched epilogue
VPC = V // N_CORES


# ----------------------------------------------------------------- host math
def _softplus64(x):
    return np.logaddexp(0.0, x.astype(np.float64))


def _log_sigmoid64(x):
    x = x.astype(np.float64)
    return np.where(x >= 0, -np.log1p(np.exp(-np.abs(x))),
                    x - np.log1p(np.exp(-np.abs(x))))


def _rne9(x):
    """Round to 9 explicit mantissa bits (exactly representable at the
    hardware's ~11-bit float32r operand truncation)."""
    x32 = np.asarray(x, np.float32)
    u = x32.view(np.uint32)
    out = ((u + np.uint32(1 << 13)) & np.uint32(0xFFFFC000)).view(np.float32)
    return np.where(np.isfinite(x32), out, x32).astype(np.float64)


def _plan_and_pack(gaussian_props, voxel_coords):
    """Compute the sparse schedule and per-core packed inputs."""
    gp = np.asarray(gaussian_props, np.float32)[0]          # (N, 28)
    vc = np.asarray(voxel_coords, np.float32)               # (V, 3)
    means = gp[:, :3].astype(np.float64)
    scales = _softplus64(gp[:, 3:6])
    inv_s = 1.0 / np.clip(scales * scales, 1e-6, None)
    logop = _log_sigmoid64(gp[:, 10])
    sem = gp[:, 11:11 + C]

    nt = V // TW
    vt = vc.reshape(nt, TW, 3)
    lo, hi = vt.min(1), vt.max(1)

    # candidate gaussians per tile: dist(mean, bbox) < 3
    m32 = gp[:, :3]
    tiles = []  # (tile_id, idx array)
    for s in range(0, nt, 1024):
        e = min(s + 1024, nt)
        cl = np.clip(m32[None, :, :], lo[s:e, None, :], hi[s:e, None, :])
        d2 = ((cl - m32[None, :, :]) ** 2).sum(-1)
        for i in range(e - s):
            idx = np.nonzero(d2[i] < R2)[0]
            if len(idx):
                tiles.append((s + i, idx))

    # sort tiles by candidate count desc, group 8 at a time (one per
    # core); each group is one slot whose width J is the group's max nb
    tiles.sort(key=lambda t: -len(t[1]))
    groups = [tiles[g:g + N_CORES] for g in range(0, len(tiles), N_CORES)]
    schedule = [(len(grp[0][1]) + BLK - 1) // BLK for grp in groups]
    S = len(schedule)                            # slots per core
    U = sum(schedule)                            # units per core

    featsa = np.zeros((N_CORES, S, 8, TW), np.float32)
    featsb = np.zeros((N_CORES, S, 8, TW), np.float32)
    lhsa = np.zeros((N_CORES, U, 8, BLK), np.float32)
    lhsb = np.zeros((N_CORES, U, 8, BLK), np.float32)
    semt = np.zeros((N_CORES, U, BLK, C + 1), bfloat16)
    # (core, slot) -> tile_id for output scatter; -1 = dummy
    slot_tile = np.full((N_CORES, S), -1, np.int64)

    # tile-local integer grid (tile = 10 y x 16 z at a single x)
    iy = np.arange(TW) // 16
    iz = np.arange(TW) % 16
    Fy = iy - 4.5                      # exact small values
    Fz = iz - 7.5
    dy = (PCR_Y1 - PCR_Y0) / (OCC[1] - 1)
    dz = (PCR_Z1 - PCR_Z0) / (OCC[2] - 1)

    for core in range(N_CORES):
        uid = 0
        for sid, J in enumerate(schedule):
            grp = groups[sid]
            if core < len(grp):
                tid, idx = grp[core]
                slot_tile[core, sid] = tid
                ctr = (0.5 * (lo[tid].astype(np.float64)
                              + hi[tid].astype(np.float64)))
                yv = vt[tid][:, 1].astype(np.float64) - ctr[1]
                zv = vt[tid][:, 2].astype(np.float64) - ctr[2]
                y2z2 = yv * yv + zv * zv
                fq_hi = _rne9(y2z2)
                # features: A uses the exact integer-grid quadratics;
                # B folds y'^2+z'^2 into a hi/lo pair (coeff 1)
                featsa[core, sid] = np.stack([
                    Fy * Fy, Fz * Fz, Fy, Fz, np.ones(TW),
                    Fy, Fz, np.ones(TW)]).astype(np.float32)
                featsb[core, sid] = np.stack([
                    fq_hi, y2z2 - fq_hi, Fy, Fz, np.ones(TW),
                    Fy, Fz, np.ones(TW)]).astype(np.float32)
                m = means[idx] - ctr[None, :]
                iv = inv_s[idx]
                n = len(idx)
                ay = 0.5 * iv[:, 1] * dy * dy
                az = 0.5 * iv[:, 2] * dz * dz
                by = -iv[:, 1] * m[:, 1] * dy
                bz = -iv[:, 2] * m[:, 2] * dz
                a0 = 0.5 * (iv[:, 1] * m[:, 1] ** 2
                            + iv[:, 2] * m[:, 2] ** 2
                            + iv[:, 0] * m[:, 0] ** 2) - logop[idx]
                by_h = _rne9(by)
                bz_h = _rne9(bz)
                a0_h = _rne9(a0)
                cA = np.zeros((8, J * BLK), np.float32)
                cA[:, :n] = np.stack([ay, az, by_h, bz_h, a0_h,
                                      by - by_h, bz - bz_h, a0 - a0_h])
                cA[4, n:] = 1e4     # padding: w = exp(-1e4) = 0
                gy = -2.0 * m[:, 1] * dy
                gz = -2.0 * m[:, 2] * dz
                g0 = (m * m).sum(1)
                gy_h = _rne9(gy)
                gz_h = _rne9(gz)
                g0_h = _rne9(g0)
                cS = np.zeros((8, J * BLK), np.float32)
                cS[:, :n] = np.stack([np.ones(n), np.ones(n), gy_h, gz_h,
                                      g0_h, gy - gy_h, gz - gz_h,
                                      g0 - g0_h])
                cS[4, n:] = 1e9     # padding: mask = 0
                # col 0 = 1 (-> ws at psum partition 0), cols 1.. = sem
                sT = np.zeros((J * BLK, C + 1), np.float32)
                sT[:n, 0] = 1.0
                sT[:n, 1:] = sem[idx]
                for j in range(J):
                    lhsa[core, uid + j] = cA[:, j*BLK:(j+1)*BLK]
                    lhsb[core, uid + j] = cS[:, j*BLK:(j+1)*BLK]
                    semt[core, uid + j] = sT[j*BLK:(j+1)*BLK].astype(bfloat16)
            # dummy slots stay all-zero (w=1 but sem=ws=0 -> out=c0)
            uid += J
    return {
        "schedule": schedule, "S": S, "U": U, "slot_tile": slot_tile,
        "featsa": featsa, "featsb": featsb,
        "lhsa": lhsa, "lhsb": lhsb, "semt": semt,
    }


# ------------------------------------------------------------- bass program
def _build_program(schedule, S, U):
    nc = bacc.Bacc("TRN2", target_bir_lowering=False, debug=False,
                   num_devices=N_CORES)

    def din(name, shape, dt=F32):
        return nc.dram_tensor(name, list(shape), dt, kind="ExternalInput").ap()

    def dout(name, shape):
        return nc.dram_tensor(name, list(shape), F32, kind="ExternalOutput").ap()

    featsr_d = din("featsr", (S, 8, TW), F32R)
    featsf_d = din("featsf", (S, 8, TW), F32R)
    lhsa_d = din("lhsa", (U, 8, BLK), F32R)
    lhsb_d = din("lhsb", (U, 8, BLK), F32R)
    semt_d = din("semt", (U, BLK, C + 1), BF16)
    w1te_d = din("w1te", (C + 1, 2 * C), BF16)   # row 0 = b1 (occ ws row ~ 1)
    w2e_d = din("w2e", (2 * C, C), BF16)
    b2_d = din("b2", (C, 1))
    slots_d = dout("slots", (S, C, TW))          # host transposes to (TW, C)

    with tile.TileContext(nc) as tc:
        with (
            tc.tile_pool(name="const", bufs=1) as constp,
            tc.tile_pool(name="wp", bufs=4) as wp,
            tc.tile_pool(name="ep", bufs=3) as ep,
            tc.tile_pool(name="psa", bufs=2, space="PSUM") as psa,
            tc.tile_pool(name="psb", bufs=2, space="PSUM") as psb,
            tc.tile_pool(name="ps2", bufs=2, space="PSUM") as ps2p,
            tc.tile_pool(name="pse", bufs=2, space="PSUM") as psep,
        ):
            # constants + all inputs in a few large DMAs
            w1te_s = constp.tile([C + 1, 2 * C], BF16, tag="w1te")
            nc.sync.dma_start(w1te_s[:], w1te_d[:])
            w2e_s = constp.tile([2 * C, C], BF16, tag="w2e")
            nc.sync.dma_start(w2e_s[:], w2e_d[:])
            b2_s = constp.tile([C, 1], F32, tag="b2")
            nc.sync.dma_start(b2_s[:], b2_d[:])
            ones_s = constp.tile([1, C + 1], BF16, tag="ones")
            nc.vector.memset(ones_s[:], 1.0)

            featsr_s = constp.tile([8, S * TW], F32R, tag="featsr")
            nc.sync.dma_start(
                featsr_s[:].rearrange("p (s f) -> p s f", f=TW),
                featsr_d.transpose([1, 0, 2]))
            featsf_s = constp.tile([8, S * TW], F32R, tag="featsf")
            nc.sync.dma_start(
                featsf_s[:].rearrange("p (s f) -> p s f", f=TW),
                featsf_d.transpose([1, 0, 2]))
            lhsa_s = constp.tile([8, U * BLK], F32R, tag="lhsa")
            lhsb_s = constp.tile([8, U * BLK], F32R, tag="lhsb")
            semt_s = constp.tile([BLK, U * (C + 1)], BF16, tag="semt")
            uc = min(16, U)   # first chunk covers the leading units
            for u0, u1 in ((0, uc), (uc, U)):
                if u0 >= u1:
                    continue
                nc.sync.dma_start(
                    lhsa_s[:, u0 * BLK:u1 * BLK]
                    .rearrange("p (u f) -> p u f", f=BLK),
                    lhsa_d[u0:u1].transpose([1, 0, 2]))
                nc.sync.dma_start(
                    lhsb_s[:, u0 * BLK:u1 * BLK]
                    .rearrange("p (u f) -> p u f", f=BLK),
                    lhsb_d[u0:u1].transpose([1, 0, 2]))
                nc.sync.dma_start(
                    semt_s[:, u0 * (C + 1):u1 * (C + 1)]
                    .rearrange("p (u f) -> p u f", f=C + 1),
                    semt_d[u0:u1].transpose([1, 0, 2]))

            # unit sequence: (slot, unit, first-in-slot, last-in-slot)
            units = []
            uid = 0
            for sid, J in enumerate(schedule):
                for j in range(J):
                    units.append((sid, uid + j, j == 0, j == J - 1))
                uid += J

            def epilogue(s0, n_slots, p2):
                # ws is p2 row 0; r broadcast via PE; the normalized ws
                # row (~1) carries b1 through w1te row 0
                W = n_slots * TW
                r_s = ep.tile([1, EB * TW], F32, tag="r")
                nc.vector.tensor_scalar_max(r_s[:, 0:W], p2[0:1, 0:W], 1e-6)
                nc.vector.reciprocal_approx_fast(r_s[:, 0:W], r_s[:, 0:W])
                rc_s = ep.tile([1, EB * TW], BF16, tag="rc")
                nc.vector.tensor_scalar_mul(rc_s[:, 0:W], r_s[:, 0:W], 1.0)
                pr = psep.tile([C + 1, EB * TW], F32, tag="pse")
                nc.tensor.matmul(pr[:, 0:W], ones_s[:], rc_s[:, 0:W],
                                 start=True, stop=True)
                rb_s = ep.tile([C + 1, EB * TW], F32, tag="rb")
                nc.scalar.activation(rb_s[:, 0:W], pr[:, 0:W], AF.Copy)
                occ_s = ep.tile([C + 1, EB * TW], BF16, tag="occ")
                nc.vector.tensor_tensor(occ_s[:, 0:W], p2[:, 0:W],
                                        rb_s[:, 0:W], op=ALU.mult)
                ph = psep.tile([2 * C, EB * TW], F32, tag="pse")
                nc.tensor.matmul(ph[:, 0:W], w1te_s[:], occ_s[:, 0:W],
                                 start=True, stop=True)
                h_s = ep.tile([2 * C, EB * TW], BF16, tag="h")
                nc.vector.tensor_scalar_max(h_s[:, 0:W], ph[:, 0:W], 0.0)
                po = psep.tile([C, EB * TW], F32, tag="pse")
                nc.tensor.matmul(po[:, 0:W], w2e_s[:], h_s[:, 0:W],
                                 start=True, stop=True)
                o_s = ep.tile([C, EB * TW], F32, tag="o")
                nc.scalar.activation(o_s[:, 0:W], po[:, 0:W], AF.Identity,
                                     bias=b2_s[:])
                nc.sync.dma_start(
                    slots_d[s0:s0 + n_slots].transpose([1, 0, 2]),
                    o_s[:, 0:W].rearrange("p (k f) -> p k f", f=TW))

            # main sparse loop: units in GN-wide groups sharing PSUM
            # banks; per-tile accumulators batched over EB slots
            p2_tiles = {}
            for p in range(0, len(units), GN):
                grp = units[p:p + GN]
                gw = len(grp) * TW
                pa2 = psa.tile([BLK, GN * TW], F32, tag="pa")
                pb2 = psb.tile([BLK, GN * TW], F32, tag="pb")
                for k, (sid, u, first, last) in enumerate(grp):
                    nc.tensor.matmul(pa2[:, bass.ts(k, TW)],
                                     lhsa_s[:, bass.ts(u, BLK)],
                                     featsr_s[:, bass.ts(sid, TW)],
                                     start=True, stop=True)
                for k, (sid, u, first, last) in enumerate(grp):
                    nc.tensor.matmul(pb2[:, bass.ts(k, TW)],
                                     lhsb_s[:, bass.ts(u, BLK)],
                                     featsf_s[:, bass.ts(sid, TW)],
                                     start=True, stop=True)
                we_s = wp.tile([BLK, GN * TW], BF16, tag="we")
                nc.scalar.activation(we_s[:, 0:gw], pa2[:, 0:gw],
                                     AF.Exp, scale=-1.0)
                w_s = wp.tile([BLK, GN * TW], BF16, tag="w")
                nc.vector.scalar_tensor_tensor(
                    w_s[:, 0:gw], pb2[:, 0:gw], float(R2), we_s[:, 0:gw],
                    op0=ALU.is_lt, op1=ALU.mult)
                for k, (sid, u, first, last) in enumerate(grp):
                    pair = sid // EB
                    if first and pair not in p2_tiles:
                        p2t = ps2p.tile([C + 1, EB * TW], F32, tag="ps2")
                        p2_tiles[pair] = p2t
                    nc.tensor.matmul(
                        p2_tiles[pair][:, bass.ts(sid % EB, TW)],
                        semt_s[:, bass.ts(u, C + 1)],
                        w_s[:, bass.ts(k, TW)],
                        start=first, stop=last, skip_group_check=True)
                    if last and (sid % EB == EB - 1 or sid == S - 1):
                        epilogue(pair * EB, sid % EB + 1,
                                 p2_tiles.pop(pair))
    return nc


# ---------------------------------------------------------------- execution
def _execute(nc, plan, W1, b1, W2, b2, trace=False, **kw):
    w1te = np.zeros((C + 1, 2 * C), np.float32)
    w1te[0] = b1
    w1te[1:] = W1.T
    consts = {
        "w1te": w1te.astype(bfloat16),
        "w2e": np.ascontiguousarray(W2.T).astype(bfloat16),
        "b2": b2.reshape(C, 1).astype(np.float32),
    }
    in_maps = []
    for core in range(N_CORES):
        m = dict(consts)
        m["featsr"] = plan["featsa"][core]
        m["featsf"] = plan["featsb"][core]
        m["lhsa"] = plan["lhsa"][core]
        m["lhsb"] = plan["lhsb"][core]
        m["semt"] = plan["semt"][core]
        in_maps.append(m)
    if not nc.is_finalized():
        nc.finalize()
    return run_bass_kernel_spmd(nc, in_maps, list(range(N_CORES)),
                                trace=trace, **kw)


def _assemble(plan, results, W1, b1, W2, b2):
    # inactive voxels: occ = 0 -> out = W2 @ relu(b1) + b2
    c0 = (W2 @ np.maximum(b1, 0.0) + b2).astype(np.float32)
    out = np.empty((V, C), np.float32)
    out[:] = c0
    slot_tile = plan["slot_tile"]
    for core in range(N_CORES):
        slots = results[core]["slots"]          # (S, C, TW)
        for sid in range(plan["S"]):
            tid = slot_tile[core, sid]
            if tid >= 0:
                out[tid * TW:(tid + 1) * TW] = slots[sid].T
    return out.reshape(1, OCC[0], OCC[1], OCC[2], C)


def run(inputs, trace=False, **kw):
    """Full pipeline; returns (output, BassKernelResults)."""
    gp = np.asarray(inputs["gaussian_props"], np.float32)
    plan = _plan_and_pack(gp, inputs["voxel_coords"])
    nc = _build_program(plan["schedule"], plan["S"], plan["U"])
    W1 = np.asarray(inputs["W1"], np.float32)
    b1 = np.asarray(inputs["b1"], np.float32)
    W2 = np.asarray(inputs["W2"], np.float32)
    b2 = np.asarray(inputs["b2"], np.float32)
    res = _execute(nc, plan, W1, b1, W2, b2, trace=trace, **kw)
    out = _assemble(plan, res.results, W1, b1, W2, b2)
    return out, res


def kernel(**inputs) -> np.ndarray:
    out, _ = run(inputs)
    return out


# revision 16
# speedup vs baseline: 2.4478x; 1.0137x over previous
"""Trainium2 Bass kernel for nn_GaussianSplattingDecoder.

Splat 2048 gaussians onto a 200x200x16 voxel grid (V=640000), then a tiny
per-voxel MLP.  Exploits the radius-3 interaction mask: gaussian means are
~N(0,1) while the grid spans +-40 in x/y, so only ~3% of voxel tiles
interact with any gaussian at all.

Strategy (8 NeuronCores, SPMD — one program, per-core data):
  - Voxel tiles of TW=160 contiguous voxels (10 y x 16 z at a single x).
    Host finds, per tile, the candidate gaussians (dist(mean, bbox) < 3),
    packs them into blocks of 128 with tile-centered quadratic-form
    coefficients so both
      A = 0.5*mahalanobis - ln(opacity)   and   B = squared distance
    are K=8 matmuls against shared per-voxel feature rows.
  - Both matmuls run single-pass float32r (the PE truncates operands to
    ~11 mantissa bits but runs at bf16 speed, 4x faster than fp32's
    LOW_HIGH two-pass mode).  Because x is constant per tile, only 5
    feature rows are needed (y'^2/z'^2 via exact small integer-grid
    features, y', z', 1); the 3 spare rows carry hi/lo-compensated
    coefficients: hi parts are pre-rounded to 9 mantissa bits (exact
    pass-through at the hardware's operand truncation, whatever its
    rounding mode) and the lo residuals ride duplicated feature rows.
    Net effective precision ~fp32 — critical for the mask compare
    (B < 9), where ~1e-2 errors flip borderline voxels and flipped
    large-scale gaussians produce O(1) occupancy errors.
  - Device, per (tile, block) unit:  w = exp(-A) * (B < 9);  then
    psum2[18, TW] += semT.T @ w  (semantics cols 0..16, col 17 = 1 -> ws).
    Units are processed in triples sharing one PSUM bank ([128, 480]) so
    exp/STT run as 3x-wide instructions (fixed per-op overhead dominates).
  - Per-tile epilogue, batched over pairs of slots ([18, 320] psum
    accumulators) and engineered off the Scalar engine: DVE computes
    r = 1/max(ws,1e-6); PE broadcasts r; DVE normalizes; MLP layer 1
    takes b1 via the ~1-valued ws row of occ (w1t row 0 = b1); DVE
    applies relu; b2 rides the scalar psum->sbuf copy.  Output stays
    [17, TW]; the host transposes.
  - All inputs load in a few large up-front DMAs (per-descriptor overhead
    on the sync queue is ~0.7us), chunked so the first units' data lands
    first.
  - Inactive voxels get the constant c0 = W2@relu(b1)+b2, filled
    host-side; active tiles are computed into slot-indexed buffers and
    scattered over the fill on the host.
  - Active tiles are sorted by block count and grouped 8 at a time (one
    per core, slot width = group max) so every core runs the identical
    static schedule; missing tiles become dummy all-zero slots which are
    numerically inert and ignored.
"""

import numpy as np
from ml_dtypes import bfloat16

import concourse.bass as bass
import concourse.bacc as bacc
import concourse.mybir as mybir
from concourse import tile
from concourse.bass_utils import run_bass_kernel_spmd

AF = mybir.ActivationFunctionType
ALU = mybir.AluOpType
F32 = mybir.dt.float32
F32R = mybir.dt.float32r
BF16 = mybir.dt.bfloat16

OCC = (200, 200, 16)
PCR_Y0, PCR_Y1 = -40.0, 40.0
PCR_Z0, PCR_Z1 = -1.0, 5.4
V = OCC[0] * OCC[1] * OCC[2]
C = 17
R2 = 9.0
TW = 160           # voxels per tile
BLK = 128          # gaussians per block
N_CORES = 8
GN = 3             # units per exp/STT group (PSUM bank holds 480 f32)
EB = 2             # slots per batched epilogue
VPC = V // N_CORES


# ----------------------------------------------------------------- host math
def _softplus64(x):
    return np.logaddexp(0.0, x.astype(np.float64))


def _log_sigmoid64(x):
    x = x.astype(np.float64)
    return np.where(x >= 0, -np.log1p(np.exp(-np.abs(x))),
                    x - np.log1p(np.exp(-np.abs(x))))


def _rne9(x):
    """Round to 9 explicit mantissa bits (exactly representable at the
    hardware's ~11-bit float32r operand truncation)."""
    x32 = np.asarray(x, np.float32)
    u = x32.view(np.uint32)
    out = ((u + np.uint32(1 << 13)) & np.uint32(0xFFFFC000)).view(np.float32)
    return np.where(np.isfinite(x32), out, x32).astype(np.float64)


def _plan_and_pack(gaussian_props, voxel_coords):
    """Compute the sparse schedule and per-core packed inputs."""
    gp = np.asarray(gaussian_props, np.float32)[0]          # (N, 28)
    vc = np.asarray(voxel_coords, np.float32)               # (V, 3)
    means = gp[:, :3].astype(np.float64)
    scales = _softplus64(gp[:, 3:6])
    inv_s = 1.0 / np.clip(scales * scales, 1e-6, None)
    logop = _log_sigmoid64(gp[:, 10])
    sem = gp[:, 11:11 + C]

    nt = V // TW
    vt = vc.reshape(nt, TW, 3)
    lo, hi = vt.min(1), vt.max(1)

    # candidate gaussians per tile: dist(mean, bbox) < 3
    m32 = gp[:, :3]
    tiles = []  # (tile_id, idx array)
    for s in range(0, nt, 1024):
        e = min(s + 1024, nt)
        cl = np.clip(m32[None, :, :], lo[s:e, None, :], hi[s:e, None, :])
        d2 = ((cl - m32[None, :, :]) ** 2).sum(-1)
        for i in range(e - s):
            idx = np.nonzero(d2[i] < R2)[0]
            if len(idx):
                tiles.append((s + i, idx))

    # sort tiles by candidate count desc, group 8 at a time (one per
    # core); each group is one slot whose width J is the group's max nb.
    # Interleave big and small slots so the small slots' epilogue latency
    # hides behind the next big slot's unit work.
    tiles.sort(key=lambda t: -len(t[1]))
    groups = [tiles[g:g + N_CORES] for g in range(0, len(tiles), N_CORES)]
    inter = []
    a, b = 0, len(groups) - 1
    while a <= b:
        inter.append(groups[a])
        a += 1
        if a <= b:
            inter.append(groups[b])
            b -= 1
    groups = inter
    schedule = [(len(grp[0][1]) + BLK - 1) // BLK for grp in groups]
    S = len(schedule)                            # slots per core
    U = sum(schedule)                            # units per core

    featsa = np.zeros((N_CORES, S, 8, TW), np.float32)
    featsb = np.zeros((N_CORES, S, 8, TW), np.float32)
    lhsa = np.zeros((N_CORES, U, 8, BLK), np.float32)
    lhsb = np.zeros((N_CORES, U, 8, BLK), np.float32)
    semt = np.zeros((N_CORES, U, BLK, C + 1), bfloat16)
    # (core, slot) -> tile_id for output scatter; -1 = dummy
    slot_tile = np.full((N_CORES, S), -1, np.int64)

    # tile-local integer grid (tile = 10 y x 16 z at a single x)
    iy = np.arange(TW) // 16
    iz = np.arange(TW) % 16
    Fy = iy - 4.5                      # exact small values
    Fz = iz - 7.5
    dy = (PCR_Y1 - PCR_Y0) / (OCC[1] - 1)
    dz = (PCR_Z1 - PCR_Z0) / (OCC[2] - 1)

    for core in range(N_CORES):
        uid = 0
        for sid, J in enumerate(schedule):
            grp = groups[sid]
            if core < len(grp):
                tid, idx = grp[core]
                slot_tile[core, sid] = tid
                ctr = (0.5 * (lo[tid].astype(np.float64)
                              + hi[tid].astype(np.float64)))
                yv = vt[tid][:, 1].astype(np.float64) - ctr[1]
                zv = vt[tid][:, 2].astype(np.float64) - ctr[2]
                y2z2 = yv * yv + zv * zv
                fq_hi = _rne9(y2z2)
                # features: A uses the exact integer-grid quadratics;
                # B folds y'^2+z'^2 into a hi/lo pair (coeff 1)
                featsa[core, sid] = np.stack([
                    Fy * Fy, Fz * Fz, Fy, Fz, np.ones(TW),
                    Fy, Fz, np.ones(TW)]).astype(np.float32)
                featsb[core, sid] = np.stack([
                    fq_hi, y2z2 - fq_hi, Fy, Fz, np.ones(TW),
                    Fy, Fz, np.ones(TW)]).astype(np.float32)
                m = means[idx] - ctr[None, :]
                iv = inv_s[idx]
                n = len(idx)
                ay = 0.5 * iv[:, 1] * dy * dy
                az = 0.5 * iv[:, 2] * dz * dz
                by = -iv[:, 1] * m[:, 1] * dy
                bz = -iv[:, 2] * m[:, 2] * dz
                a0 = 0.5 * (iv[:, 1] * m[:, 1] ** 2
                            + iv[:, 2] * m[:, 2] ** 2
                            + iv[:, 0] * m[:, 0] ** 2) - logop[idx]
                by_h = _rne9(by)
                bz_h = _rne9(bz)
                a0_h = _rne9(a0)
                cA = np.zeros((8, J * BLK), np.float32)
                cA[:, :n] = np.stack([ay, az, by_h, bz_h, a0_h,
                                      by - by_h, bz - bz_h, a0 - a0_h])
                cA[4, n:] = 1e4     # padding: w = exp(-1e4) = 0
                gy = -2.0 * m[:, 1] * dy
                gz = -2.0 * m[:, 2] * dz
                g0 = (m * m).sum(1)
                gy_h = _rne9(gy)
                gz_h = _rne9(gz)
                g0_h = _rne9(g0)
                cS = np.zeros((8, J * BLK), np.float32)
                cS[:, :n] = np.stack([np.ones(n), np.ones(n), gy_h, gz_h,
                                      g0_h, gy - gy_h, gz - gz_h,
                                      g0 - g0_h])
                cS[4, n:] = 1e9     # padding: mask = 0
                # col 0 = 1 (-> ws at psum partition 0), cols 1.. = sem
                sT = np.zeros((J * BLK, C + 1), np.float32)
                sT[:n, 0] = 1.0
                sT[:n, 1:] = sem[idx]
                for j in range(J):
                    lhsa[core, uid + j] = cA[:, j*BLK:(j+1)*BLK]
                    lhsb[core, uid + j] = cS[:, j*BLK:(j+1)*BLK]
                    semt[core, uid + j] = sT[j*BLK:(j+1)*BLK].astype(bfloat16)
            # dummy slots stay all-zero (w=1 but sem=ws=0 -> out=c0)
            uid += J
    return {
        "schedule": schedule, "S": S, "U": U, "slot_tile": slot_tile,
        "featsa": featsa, "featsb": featsb,
        "lhsa": lhsa, "lhsb": lhsb, "semt": semt,
    }


# ------------------------------------------------------------- bass program
def _build_program(schedule, S, U):
    nc = bacc.Bacc("TRN2", target_bir_lowering=False, debug=False,
                   num_devices=N_CORES)

    def din(name, shape, dt=F32):
        return nc.dram_tensor(name, list(shape), dt, kind="ExternalInput").ap()

    def dout(name, shape):
        return nc.dram_tensor(name, list(shape), F32, kind="ExternalOutput").ap()

    featsr_d = din("featsr", (S, 8, TW), F32R)
    featsf_d = din("featsf", (S, 8, TW), F32R)
    lhsa_d = din("lhsa", (U, 8, BLK), F32R)
    lhsb_d = din("lhsb", (U, 8, BLK), F32R)
    semt_d = din("semt", (U, BLK, C + 1), BF16)
    w1te_d = din("w1te", (C + 1, 2 * C), BF16)   # row 0 = b1 (occ ws row ~ 1)
    w2e_d = din("w2e", (2 * C, C), BF16)
    b2_d = din("b2", (C, 1))
    slots_d = dout("slots", (S, C, TW))          # host transposes to (TW, C)

    with tile.TileContext(nc) as tc:
        with (
            tc.tile_pool(name="const", bufs=1) as constp,
            tc.tile_pool(name="wp", bufs=4) as wp,
            tc.tile_pool(name="ep", bufs=3) as ep,
            tc.tile_pool(name="psa", bufs=2, space="PSUM") as psa,
            tc.tile_pool(name="psb", bufs=2, space="PSUM") as psb,
            tc.tile_pool(name="ps2", bufs=2, space="PSUM") as ps2p,
            tc.tile_pool(name="pse", bufs=2, space="PSUM") as psep,
        ):
            # constants + all inputs in a few large DMAs
            w1te_s = constp.tile([C + 1, 2 * C], BF16, tag="w1te")
            nc.sync.dma_start(w1te_s[:], w1te_d[:])
            w2e_s = constp.tile([2 * C, C], BF16, tag="w2e")
            nc.sync.dma_start(w2e_s[:], w2e_d[:])
            b2_s = constp.tile([C, 1], F32, tag="b2")
            nc.sync.dma_start(b2_s[:], b2_d[:])
            ones_s = constp.tile([1, C + 1], BF16, tag="ones")
            nc.vector.memset(ones_s[:], 1.0)

            featsr_s = constp.tile([8, S * TW], F32R, tag="featsr")
            nc.sync.dma_start(
                featsr_s[:].rearrange("p (s f) -> p s f", f=TW),
                featsr_d.transpose([1, 0, 2]))
            featsf_s = constp.tile([8, S * TW], F32R, tag="featsf")
            nc.sync.dma_start(
                featsf_s[:].rearrange("p (s f) -> p s f", f=TW),
                featsf_d.transpose([1, 0, 2]))
            # unit data in two chunks as SEPARATE tiles (a shared tile
            # would make the first matmul depend on the last DMA)
            cut = min(sum(schedule[:2]), U)
            lhsa_t = []
            lhsb_t = []
            semt_t = []
            for ci, (u0, u1) in enumerate(((0, cut), (cut, U))):
                if u0 >= u1:
                    continue
                un = u1 - u0
                la = constp.tile([8, un * BLK], F32R, tag=f"lhsa{ci}")
                nc.sync.dma_start(
                    la[:].rearrange("p (u f) -> p u f", f=BLK),
                    lhsa_d[u0:u1].transpose([1, 0, 2]))
                lb = constp.tile([8, un * BLK], F32R, tag=f"lhsb{ci}")
                nc.sync.dma_start(
                    lb[:].rearrange("p (u f) -> p u f", f=BLK),
                    lhsb_d[u0:u1].transpose([1, 0, 2]))
                st = constp.tile([BLK, un * (C + 1)], BF16, tag=f"semt{ci}")
                nc.sync.dma_start(
                    st[:].rearrange("p (u f) -> p u f", f=C + 1),
                    semt_d[u0:u1].transpose([1, 0, 2]))
                lhsa_t.append(la)
                lhsb_t.append(lb)
                semt_t.append(st)

            def lhsa_sl(u):
                return (lhsa_t[0][:, bass.ts(u, BLK)] if u < cut
                        else lhsa_t[1][:, bass.ts(u - cut, BLK)])

            def lhsb_sl(u):
                return (lhsb_t[0][:, bass.ts(u, BLK)] if u < cut
                        else lhsb_t[1][:, bass.ts(u - cut, BLK)])

            def semt_sl(u):
                return (semt_t[0][:, bass.ts(u, C + 1)] if u < cut
                        else semt_t[1][:, bass.ts(u - cut, C + 1)])

            # unit sequence: (slot, unit, first-in-slot, last-in-slot)
            units = []
            uid = 0
            for sid, J in enumerate(schedule):
                for j in range(J):
                    units.append((sid, uid + j, j == 0, j == J - 1))
                uid += J

            def epilogue(s0, n_slots, p2):
                # ws is p2 row 0; r broadcast via PE; the normalized ws
                # row (~1) carries b1 through w1te row 0
                W = n_slots * TW
                r_s = ep.tile([1, EB * TW], F32, tag="r")
                nc.vector.tensor_scalar_max(r_s[:, 0:W], p2[0:1, 0:W], 1e-6)
                nc.vector.reciprocal_approx_fast(r_s[:, 0:W], r_s[:, 0:W])
                rc_s = ep.tile([1, EB * TW], BF16, tag="rc")
                nc.vector.tensor_scalar_mul(rc_s[:, 0:W], r_s[:, 0:W], 1.0)
                pr = psep.tile([C + 1, EB * TW], F32, tag="pse")
                nc.tensor.matmul(pr[:, 0:W], ones_s[:], rc_s[:, 0:W],
                                 start=True, stop=True)
                rb_s = ep.tile([C + 1, EB * TW], F32, tag="rb")
                nc.scalar.activation(rb_s[:, 0:W], pr[:, 0:W], AF.Copy)
                occ_s = ep.tile([C + 1, EB * TW], BF16, tag="occ")
                nc.vector.tensor_tensor(occ_s[:, 0:W], p2[:, 0:W],
                                        rb_s[:, 0:W], op=ALU.mult)
                ph = psep.tile([2 * C, EB * TW], F32, tag="pse")
                nc.tensor.matmul(ph[:, 0:W], w1te_s[:], occ_s[:, 0:W],
                                 start=True, stop=True)
                h_s = ep.tile([2 * C, EB * TW], BF16, tag="h")
                nc.vector.tensor_scalar_max(h_s[:, 0:W], ph[:, 0:W], 0.0)
                po = psep.tile([C, EB * TW], F32, tag="pse")
                nc.tensor.matmul(po[:, 0:W], w2e_s[:], h_s[:, 0:W],
                                 start=True, stop=True)
                o_s = ep.tile([C, EB * TW], F32, tag="o")
                nc.scalar.activation(o_s[:, 0:W], po[:, 0:W], AF.Identity,
                                     bias=b2_s[:])
                nc.sync.dma_start(
                    slots_d[s0:s0 + n_slots].transpose([1, 0, 2]),
                    o_s[:, 0:W].rearrange("p (k f) -> p k f", f=TW))

            # main sparse loop: units in GN-wide groups sharing PSUM
            # banks; per-tile accumulators batched over EB slots
            p2_tiles = {}
            for p in range(0, len(units), GN):
                grp = units[p:p + GN]
                gw = len(grp) * TW
                pa2 = psa.tile([BLK, GN * TW], F32, tag="pa")
                pb2 = psb.tile([BLK, GN * TW], F32, tag="pb")
                for k, (sid, u, first, last) in enumerate(grp):
                    nc.tensor.matmul(pa2[:, bass.ts(k, TW)],
                                     lhsa_sl(u),
                                     featsr_s[:, bass.ts(sid, TW)],
                                     start=True, stop=True)
                for k, (sid, u, first, last) in enumerate(grp):
                    nc.tensor.matmul(pb2[:, bass.ts(k, TW)],
                                     lhsb_sl(u),
                                     featsf_s[:, bass.ts(sid, TW)],
                                     start=True, stop=True)
                we_s = wp.tile([BLK, GN * TW], BF16, tag="we")
                nc.scalar.activation(we_s[:, 0:gw], pa2[:, 0:gw],
                                     AF.Exp, scale=-1.0)
                w_s = wp.tile([BLK, GN * TW], BF16, tag="w")
                nc.vector.scalar_tensor_tensor(
                    w_s[:, 0:gw], pb2[:, 0:gw], float(R2), we_s[:, 0:gw],
                    op0=ALU.is_lt, op1=ALU.mult)
                for k, (sid, u, first, last) in enumerate(grp):
                    pair = sid // EB
                    if first and pair not in p2_tiles:
                        p2t = ps2p.tile([C + 1, EB * TW], F32, tag="ps2")
                        p2_tiles[pair] = p2t
                    nc.tensor.matmul(
                        p2_tiles[pair][:, bass.ts(sid % EB, TW)],
                        semt_sl(u),
                        w_s[:, bass.ts(k, TW)],
                        start=first, stop=last, skip_group_check=True)
                    if last and (sid % EB == EB - 1 or sid == S - 1):
                        epilogue(pair * EB, sid % EB + 1,
                                 p2_tiles.pop(pair))
    return nc


# ---------------------------------------------------------------- execution
def _execute(nc, plan, W1, b1, W2, b2, trace=False, **kw):
    w1te = np.zeros((C + 1, 2 * C), np.float32)
    w1te[0] = b1
    w1te[1:] = W1.T
    consts = {
        "w1te": w1te.astype(bfloat16),
        "w2e": np.ascontiguousarray(W2.T).astype(bfloat16),
        "b2": b2.reshape(C, 1).astype(np.float32),
    }
    in_maps = []
    for core in range(N_CORES):
        m = dict(consts)
        m["featsr"] = plan["featsa"][core]
        m["featsf"] = plan["featsb"][core]
        m["lhsa"] = plan["lhsa"][core]
        m["lhsb"] = plan["lhsb"][core]
        m["semt"] = plan["semt"][core]
        in_maps.append(m)
    if not nc.is_finalized():
        nc.finalize()
    return run_bass_kernel_spmd(nc, in_maps, list(range(N_CORES)),
                                trace=trace, **kw)


def _assemble(plan, results, W1, b1, W2, b2):
    # inactive voxels: occ = 0 -> out = W2 @ relu(b1) + b2
    c0 = (W2 @ np.maximum(b1, 0.0) + b2).astype(np.float32)
    out = np.empty((V, C), np.float32)
    out[:] = c0
    slot_tile = plan["slot_tile"]
    for core in range(N_CORES):
        slots = results[core]["slots"]          # (S, C, TW)
        for sid in range(plan["S"]):
            tid = slot_tile[core, sid]
            if tid >= 0:
                out[tid * TW:(tid + 1) * TW] = slots[sid].T
    return out.reshape(1, OCC[0], OCC[1], OCC[2], C)


def run(inputs, trace=False, **kw):
    """Full pipeline; returns (output, BassKernelResults)."""
    gp = np.asarray(inputs["gaussian_props"], np.float32)
    plan = _plan_and_pack(gp, inputs["voxel_coords"])
    nc = _build_program(plan["schedule"], plan["S"], plan["U"])
    W1 = np.asarray(inputs["W1"], np.float32)
    b1 = np.asarray(inputs["b1"], np.float32)
    W2 = np.asarray(inputs["W2"], np.float32)
    b2 = np.asarray(inputs["b2"], np.float32)
    res = _execute(nc, plan, W1, b1, W2, b2, trace=trace, **kw)
    out = _assemble(plan, res.results, W1, b1, W2, b2)
    return out, res


def kernel(**inputs) -> np.ndarray:
    out, _ = run(inputs)
    return out
